# revision 7
# baseline (speedup 1.0000x reference)
"""MultiHeadDiffAttn Trainium2 kernel (v3, batched-epilogue path).

Sharding: 8 cores = 4-way data parallel over batch x 2-way tensor parallel
over heads (8 v-heads / 16 half-heads per core).  Each core computes its
batch's qkv projection restricted to its head group, differential attention
with per-half-head softmax, head RMS norm, and a partial output projection
(its 512 rows of w_proj).  Host sums the two partial projections per batch.

v3 changes over the v2 baseline (198us):
  - The per-(head,tj) normalize/combine epilogue (~450 small DVE ops, ~95us
    busy) is algebraically reworked to oh = U0 - lam'*U1 with
    lam' = lam*den0/den1 (RMS norm is scale-invariant; the eps term is kept
    exact via bias = eps*den0^2) and batched over all 8 tj of a head with
    stride-0 broadcast APs: ~9 DVE ops per head.
  - exp and the causal affine_select run e-batched over one [128, 2, T-t0]
    AP (s_big/es tiles hold both half-heads), halving ACT/GpSimd fixed
    overhead; the S^T matmul also emits one e-batched instruction.
  - q chunks scatter straight from PSUM into qTp (the q half of qkT was
    written but never read); input DMA issue is spread across the sync and
    scalar queues and ordered so the first matmul's operands land first;
    the qTp zero-fill is split across vector/gpsimd.
  - y is stored f16 (host sums partials in f32), output DMA goes on the
    scalar queue, outcat transposes are split across two queues with the
    heads-0-3 half transposed as soon as the first RMS batch completes.
"""

import math
from contextlib import ExitStack

import numpy as np

import concourse.bass as bass
import concourse.tile as tile
from concourse import mybir
from concourse.bass_utils import run_bass_kernel_spmd

# The deployed walrus rejects instructions carrying more than one sync wait
# ("Too many sync wait commands" in setupSyncWait).  Legalize at the BIR-JSON
# level: for every instruction with >1 wait, hoist the extra waits onto NoOp
# instructions inserted just before it on the same engine (engine streams are
# in-order, so semantics are identical).
_MAX_WAITS = 1


def _legalize_sync_waits(d):
    for f in d.get("functions", []):
        for bb in f.get("blocks", []):
            out = []
            for inst in bb["instructions"]:
                si = inst.get("sync_info")
                waits = (si or {}).get("on_wait") or []
                if len(waits) > _MAX_WAITS:
                    extra = waits[: len(waits) - _MAX_WAITS]
                    keep = waits[len(waits) - _MAX_WAITS :]
                    for j in range(0, len(extra), _MAX_WAITS):
                        nop = {
                            "engine": inst["engine"],
                            "ins": [],
                            "outs": [],
                            "name": f"{inst['name']}-lw{j}",
                            "opcode": "NoOp",
                            "sync_info": {
                                "on_wait": extra[j : j + _MAX_WAITS],
                                "on_update": [],
                            },
                        }
                        if "debug" in inst:
                            nop["debug"] = inst["debug"]
                        out.append(nop)
                    si["on_wait"] = keep
                out.append(inst)
            bb["instructions"] = out
    return d


_orig_to_json_bytes = bass.Bass.to_json_bytes


def _patched_to_json_bytes(self, *a, **kw):
    import json as _json

    raw = _orig_to_json_bytes(self, *a, **kw)
    return _json.dumps(_legalize_sync_waits(_json.loads(raw))).encode()


bass.Bass.to_json_bytes = _patched_to_json_bytes

F32 = mybir.dt.float32
F16 = mybir.dt.float16

B, T, C = 4, 1024, 1024
H_TOT = 16  # total v-heads
HD = 32  # half-head dim
DV = 64  # v-head dim
G = 2  # head groups (tensor parallel)
HPG = H_TOT // G  # 8 v-heads per core
COLS = 1024  # q cols + k cols per group
LAMBDA_INIT = 0.8 - 0.6 * math.exp(-0.3 * (1 - 1))  # 0.2
EPS = 1e-5
N_CORES = 8

NT = T // 128  # 8 t-tiles
NKC = C // 128  # 8 contraction chunks


def _emit(ctx: ExitStack, tc: tile.TileContext, xT, w_qk, w_v, w_p, lam, y):
    nc = tc.nc
    AluOp = mybir.AluOpType
    Act = mybir.ActivationFunctionType

    const = ctx.enter_context(tc.tile_pool(name="const", bufs=1))
    lam_sb = const.tile([128, 1], F32)
    nc.sync.dma_start(out=lam_sb, in_=lam[:])

    big = ctx.enter_context(tc.tile_pool(name="big", bufs=1))
    kT_sb = big.tile([128, 4, T], F16)  # row-chunks of k^T [512, T]
    v_sb = big.tile([128, NT, HPG, 128], F16)  # [s-chunk][head][dv | ones]
    outcat_sb = big.tile([128, NT, HPG * DV], F16)  # [t-chunk][512]
    outcatT_sb = big.tile([128, 4, T], F16)  # row-chunks of [512, T]
    wp_sb = big.tile([128, 4, C], F16)
    # per-half-head q, zero-padded to K=128: data lives at the same 32-row
    # strip as that half-head's k rows inside its kT chunk, so the S^T
    # matmul can contract over the full 128 partitions at full stream rate
    # (the other half-heads' k rows meet zero q rows).
    qTp_sb = big.tile([128, 2 * HPG, T], F16)

    nc.vector.memset(qTp_sb[:, 0:8, :], 0.0)
    nc.gpsimd.memset(qTp_sb[:, 8:16, :], 0.0)
    nc.gpsimd.memset(v_sb[:, :, :, DV : DV + 1], 1.0)  # softmax-denominator col
    es_pool = ctx.enter_context(tc.tile_pool(name="es", bufs=3))

    # ---------------- phase 1+2: qkv projections ----------------
    with (
        tc.tile_pool(name="xw", bufs=1) as xw,
        tc.tile_pool(name="mmps", bufs=4, space="PSUM") as mmps,
    ):
        xT_sb = xw.tile([128, NKC, T], F16)
        wqk_sb = xw.tile([128, NKC, COLS], F16)
        wv_sb = xw.tile([128, NKC, 512], F16)

        xT_r = xT[:].rearrange("(c p) t -> p c t", p=128)
        wqk_r = w_qk[:].rearrange("(c p) m -> p c m", p=128)

        def load_wqk(cc):
            nc.sync.dma_start(
                out=wqk_sb[:, :, cc * 128 : (cc + 1) * 128],
                in_=wqk_r[:, :, cc * 128 : (cc + 1) * 128],
            )

        # order feeds the first matmuls first; xT/wv/wp issue on the scalar
        # queue so they don't serialize behind the wqk issues on sync.
        load_wqk(0)
        nc.scalar.dma_start(out=xT_sb[:, :, 0:512], in_=xT_r[:, :, 0:512])
        load_wqk(1)
        load_wqk(2)
        nc.scalar.dma_start(out=xT_sb[:, :, 512:1024], in_=xT_r[:, :, 512:1024])
        for cc in range(3, 8):
            load_wqk(cc)
        nc.scalar.dma_start(
            out=wv_sb, in_=w_v[:].rearrange("(c p) m -> p c m", p=128)
        )
        nc.scalar.dma_start(
            out=wp_sb, in_=w_p[:].rearrange("(c p) m -> p c m", p=128)
        )

        # chunk cc of [q^T; k^T] = w_qk[:, cc-block].T @ x^T
        for cc in range(8):
            for nh in range(2):
                ps = mmps.tile([128, 1024], F32, tag="mmps", name=f"qk{cc}{nh}")[:, 0:512]
                for kc in range(NKC):
                    nc.tensor.matmul(
                        ps,
                        wqk_sb[:, kc, cc * 128 : (cc + 1) * 128],
                        xT_sb[:, kc, nh * 512 : (nh + 1) * 512],
                        start=(kc == 0),
                        stop=(kc == NKC - 1),
                    )
                if cc < 4:  # q chunk: scatter the 4 half-heads into qTp
                    for j in range(4):
                        hh = cc * 4 + j
                        nc.vector.tensor_copy(
                            out=qTp_sb[
                                j * 32 : (j + 1) * 32,
                                hh,
                                nh * 512 : (nh + 1) * 512,
                            ],
                            in_=ps[j * 32 : (j + 1) * 32, :],
                        )
                else:  # k chunk
                    nc.vector.tensor_copy(
                        out=kT_sb[:, cc - 4, nh * 512 : (nh + 1) * 512], in_=ps
                    )

        # prebake head 0, s=0..1: S/exp/mask run during the v projection
        prebaked = []
        for s in range(2):
            t0 = 128 * s
            es_t = es_pool.tile([128, 2, T], F16, tag="es", name=f"esp_{s}")
            for e in range(2):
                sps = mmps.tile([128, 1024], F32, tag="mmps", name=f"sp{e}_{s}")
                for c0, c1 in [(t0, 512), (512, 1024)]:
                    nc.tensor.matmul(
                        sps[:, c0:c1],
                        kT_sb[:, 0, t0 : t0 + 128],
                        qTp_sb[:, e, c0:c1],
                        start=True,
                        stop=True,
                    )
                nc.scalar.activation(
                    out=es_t[:, e, t0:T],
                    in_=sps[:, t0:T],
                    func=Act.Exp,
                    scale=1.0 / 32.0,
                )
            nc.gpsimd.affine_select(
                out=es_t[:, :, t0 : t0 + 128],
                in_=es_t[:, :, t0 : t0 + 128],
                pattern=[[0, 2], [1, 128]],
                compare_op=AluOp.is_ge,
                fill=0.0,
                base=0,
                channel_multiplier=-1,
            )
            prebaked.append((s, es_t))

        # v[t-block, :] = x @ w_v ; scatter heads into v_sb (col 64 = ones)
        for tt in range(NT):
            ps = mmps.tile([128, 1024], F32, tag="mmps", name=f"v{tt}")[:, 0:512]
            for kc in range(NKC):
                nc.tensor.matmul(
                    ps,
                    xT_sb[:, kc, tt * 128 : (tt + 1) * 128],
                    wv_sb[:, kc, :],
                    start=(kc == 0),
                    stop=(kc == NKC - 1),
                )
            nc.vector.tensor_copy(
                out=v_sb[:, tt, :, 0:DV],
                in_=ps[:].rearrange("p (h d) -> p h d", h=HPG),
            )

    # ---------------- phase 3: differential attention ----------------
    with (
        tc.tile_pool(name="sps", bufs=1, space="PSUM") as s_pool,
        tc.tile_pool(name="ups", bufs=1, space="PSUM") as u_pool,
        tc.tile_pool(name="comb", bufs=2) as comb,
        tc.tile_pool(name="ohp", bufs=1, space="SBUF") as ohp,
    ):
        oh_tiles = {}
        # per-(h,tj) rsqrt input: ssq/64 + eps*den0^2
        ssq_all = ohp.tile([128, HPG, NT], F32, tag="ssqall", name="ssqall")

        def emit_av(h, s, e, es_t, u_big):
            # U[t-block, dv|den] += expS^T[s-block, t-block].T @ v_aug[s-block]
            for tj in range(s, NT):
                nc.tensor.matmul(
                    u_big[:, e, tj, 0 : DV + 1],
                    es_t[:, e, tj * 128 : (tj + 1) * 128],
                    v_sb[:, s, h, 0 : DV + 1],
                    start=(s == 0 and tj % 4 == 0),
                    stop=(s == tj and tj % 4 == 3),
                )

        # ---- RMS in two batches: heads 0-3 overlap attention of heads 4-7
        def emit_rms(h_lo, h_hi):
            w = (h_hi - h_lo) * NT
            rstd = comb.tile([128, w], F32, tag=f"rstd{h_lo}", name=f"rstd{h_lo}")
            nc.scalar.activation(
                out=rstd, in_=ssq_all[:, h_lo:h_hi, :], func=Act.Sqrt
            )
            nc.vector.reciprocal(out=rstd, in_=rstd)
            for h in range(h_lo, h_hi):
                c0 = (h - h_lo) * NT
                nc.vector.tensor_mul(
                    outcat_sb[:, :, h * DV : (h + 1) * DV],
                    oh_tiles[h],
                    rstd[:, c0 : c0 + NT][:, :, None].broadcast_to([128, NT, DV]),
                )
            # heads 0-3 rows of outcat^T can transpose before heads 4-7 land
            lo2, hi2 = (0, 256) if h_hi < HPG else (256, 512)
            for tj in range(NT):
                nc.sync.dma_start_transpose(
                    out=outcatT_sb[:, lo2 // 128 : hi2 // 128, tj * 128 : (tj + 1) * 128],
                    in_=outcat_sb[:, tj, lo2:hi2],
                )

        for h in range(HPG):
            kc_ = h // 2
            s_big = s_pool.tile([128, 2, T], F32, tag="sb", name=f"s_{h}")
            u_big = u_pool.tile([128, 2, NT, 128], F32, tag="ub", name=f"u_{h}")
            if h == 0:
                # s=0 AVs immediately; s=1 becomes the delayed pair
                s0_, es0_ = prebaked[0]
                emit_av(h, s0_, 0, es0_, u_big)
                emit_av(h, s0_, 1, es0_, u_big)
                prev = prebaked[1]
                s_start = 2
            else:
                prev = None
                s_start = 0
            for s in range(s_start, NT):
                t0 = 128 * s
                chunks = [(t0, 512), (512, 1024)] if s < 4 else [(t0, 1024)]
                es_t = es_pool.tile([128, 2, T], F16, tag="es", name=f"es_{h}_{s}")
                for c0, c1 in chunks:
                    for e in range(2):
                        nc.tensor.matmul(
                            s_big[:, e, c0:c1],
                            kT_sb[:, kc_, t0 : t0 + 128],
                            qTp_sb[:, 2 * h + e, c0:c1],
                            start=True,
                            stop=True,
                        )
                if prev is not None:
                    ps_, pes_ = prev
                    emit_av(h, ps_, 0, pes_, u_big)
                    emit_av(h, ps_, 1, pes_, u_big)
                nc.scalar.activation(
                    out=es_t[:, :, t0:T],
                    in_=s_big[:, :, t0:T],
                    func=Act.Exp,
                    scale=1.0 / 32.0,
                )
                # causal mask inside the diagonal block: keep t >= s, both e
                nc.gpsimd.affine_select(
                    out=es_t[:, :, t0 : t0 + 128],
                    in_=es_t[:, :, t0 : t0 + 128],
                    pattern=[[0, 2], [1, 128]],
                    compare_op=AluOp.is_ge,
                    fill=0.0,
                    base=0,
                    channel_multiplier=-1,
                )
                prev = (s, es_t)
            ps_, pes_ = prev
            emit_av(h, ps_, 0, pes_, u_big)
            emit_av(h, ps_, 1, pes_, u_big)

            # ---- batched normalize/combine epilogue --------------------
            # oh = U0/den0 - lam*U1/den1; RMS norm is scale-invariant, so
            # normalize(oh) == normalize(U0 - lam') with lam' = lam*den0/den1
            # and the eps term handled exactly via bias = eps*den0^2:
            #   oh/rms(oh) = oh' * rsqrt(mean(oh'^2) + eps*den0^2)
            ub = u_big[:]  # [128, 2, NT, 128]
            U0 = ub[:, 0, :, 0:DV]
            U1 = ub[:, 1, :, 0:DV]
            d0 = ub[:, 0, :, DV : DV + 1]
            d1 = ub[:, 1, :, DV : DV + 1]
            rden = comb.tile([128, NT], F32, tag="rd", name=f"rd{h}")
            nc.vector.reciprocal(out=rden, in_=d1)
            # d0s = sqrt(eps)*den0 in SBUF (DVE ops may read only one PSUM
            # operand); lam_sb carries lam/sqrt(eps) so lamp = lam*den0/den1.
            d0s = comb.tile([128, NT], F32, tag="d0", name=f"d0{h}")
            nc.vector.tensor_scalar(
                out=d0s, in0=d0, scalar1=math.sqrt(EPS), scalar2=None, op0=AluOp.mult
            )
            lamp = comb.tile([128, NT], F32, tag="lp", name=f"lp{h}")
            nc.vector.scalar_tensor_tensor(
                out=lamp,
                in0=d0s,
                scalar=lam_sb[:],
                in1=rden,
                op0=AluOp.mult,
                op1=AluOp.mult,
            )
            tmp = comb.tile([128, NT, DV], F32, tag="tmp", name=f"tmp{h}")
            nc.vector.tensor_mul(
                tmp, U1, lamp[:, :, None].broadcast_to([128, NT, DV])
            )
            oh = ohp.tile([128, NT, DV], F32, tag=f"oh{h}", name=f"oh{h}")
            nc.vector.tensor_sub(oh, U0, tmp)
            sq = comb.tile([128, NT, DV], F32, tag="sq", name=f"sq{h}")
            nc.vector.tensor_mul(sq, oh, oh)
            nc.vector.tensor_reduce(
                out=ssq_all[:, h, :], in_=sq, axis=mybir.AxisListType.X, op=AluOp.add
            )
            bias = comb.tile([128, NT], F32, tag="bi", name=f"bi{h}")
            nc.vector.tensor_mul(bias, d0s, d0s)  # = eps*den0^2
            nc.vector.scalar_tensor_tensor(
                out=ssq_all[:, h, :],
                in0=ssq_all[:, h, :],
                scalar=1.0 / DV,
                in1=bias,
                op0=AluOp.mult,
                op1=AluOp.add,
            )
            oh_tiles[h] = oh
            if h == 3:
                emit_rms(0, 4)

        emit_rms(4, HPG)

    # ---------------- phase 4+5: output projection ----------------
    with (
        tc.tile_pool(name="pps", bufs=4, space="PSUM") as pps,
        tc.tile_pool(name="yout", bufs=2) as yout,
    ):
        for tt in range(NT):
            yt = yout.tile([128, C], F16, tag="yt", name=f"y{tt}")
            for nh in range(2):
                ps = pps.tile([128, 512], F32, tag="pp", name=f"pp{tt}{nh}")
                for rr in range(4):
                    nc.tensor.matmul(
                        ps,
                        outcatT_sb[:, rr, tt * 128 : (tt + 1) * 128],
                        wp_sb[:, rr, nh * 512 : (nh + 1) * 512],
                        start=(rr == 0),
                        stop=(rr == 3),
                    )
                nc.vector.tensor_copy(out=yt[:, nh * 512 : (nh + 1) * 512], in_=ps)
            nc.scalar.dma_start(out=y[tt * 128 : (tt + 1) * 128, :], in_=yt)


def build_nc():
    nc = bass.Bass()
    xT = nc.declare_dram_parameter("xT", [C, T], F16, isOutput=False)
    w_qk = nc.declare_dram_parameter("w_qk", [C, COLS], F16, isOutput=False)
    w_v = nc.declare_dram_parameter("w_v", [C, 512], F16, isOutput=False)
    w_p = nc.declare_dram_parameter("w_p", [512, C], F16, isOutput=False)
    lam = nc.declare_dram_parameter("lam", [128, 1], F32, isOutput=False)
    y = nc.declare_dram_parameter("y", [T, C], F16, isOutput=True)
    with tile.TileContext(nc) as tc:
        with ExitStack() as ctx:
            _emit(ctx, tc, xT, w_qk, w_v, w_p, lam, y)
    return nc


_NC = None


def _get_nc():
    global _NC
    if _NC is None:
        _NC = build_nc()
    return _NC


def make_in_maps(x, w_attn, w_proj, lambda_q1, lambda_q2, lambda_k1, lambda_k2, gamma):
    x = np.asarray(x, np.float32)
    w_attn = np.asarray(w_attn, np.float32)
    w_proj = np.asarray(w_proj, np.float32)
    lam1 = np.exp(np.sum(np.float32(lambda_q1) * np.float32(lambda_k1), dtype=np.float32))
    lam2 = np.exp(np.sum(np.float32(lambda_q2) * np.float32(lambda_k2), dtype=np.float32))
    lam_full = np.float32(lam1 - lam2 + LAMBDA_INIT)
    # kernel multiplies lam by d0s = sqrt(eps)*den0, so pre-divide here
    lam_tile = np.full((128, 1), lam_full / np.float32(math.sqrt(EPS)), np.float32)
    # fold gamma * (1 - lambda_init) into w_proj rows
    scale = np.tile(np.asarray(gamma, np.float32), H_TOT) * np.float32(1.0 - LAMBDA_INIT)
    w_p_full = (w_proj * scale[:, None]).astype(np.float16)

    in_maps = []
    for core in range(N_CORES):
        b, g = core // G, core % G
        in_maps.append(
            {
                "xT": np.ascontiguousarray(x[b].T.astype(np.float16)),
                "w_qk": np.ascontiguousarray(
                    np.concatenate(
                        [
                            w_attn[:, g * 512 : (g + 1) * 512],
                            w_attn[:, C + g * 512 : C + (g + 1) * 512],
                        ],
                        axis=1,
                    ).astype(np.float16)
                ),
                "w_v": np.ascontiguousarray(
                    w_attn[:, 2 * C + g * 512 : 2 * C + (g + 1) * 512].astype(
                        np.float16
                    )
                ),
                "w_p": np.ascontiguousarray(w_p_full[g * 512 : (g + 1) * 512, :]),
                "lam": lam_tile,
            }
        )
    return in_maps


def assemble(results):
    y = np.empty((B, T, C), np.float32)
    for b in range(B):
        y[b] = results[b * G]["y"].astype(np.float32) + results[b * G + 1][
            "y"
        ].astype(np.float32)
    return y


def kernel(**inputs) -> np.ndarray:
    nc = _get_nc()
    in_maps = make_in_maps(**inputs)
    res = run_bass_kernel_spmd(nc, in_maps, list(range(N_CORES)))
    return assemble(res.results)


# revision 8
# speedup vs baseline: 1.3719x; 1.3719x over previous
"""MultiHeadDiffAttn Trainium2 kernel (v4, lookahead producer/consumer).

Sharding: 8 cores = 4-way data parallel over batch x 2-way tensor parallel
over heads (8 v-heads / 16 half-heads per core).  Each core computes its
batch's qkv projection restricted to its head group, differential attention
with per-half-head softmax, head RMS norm, and a partial output projection
(its 512 rows of w_proj).  Host sums the two partial projections per batch.

Structure (what the measured traces drove):
  - ACT (exp) is the attention-phase bottleneck while the qkv phase leaves
    it idle, and a single S buffer serializes S(s+1) behind exp(s) (WAR).
    So S/exp/mask production is decoupled from AV consumption: S goes
    through two half-size PSUM buffers (2x [128,2,512], chunks alternate),
    exp writes ragged es tiles (only cols [t0:T]), and the producer for
    head h+2 is interleaved into head h's AV slot.  Heads 0-1 produce
    entirely inside the qkv phase (interleaved with the v-projection), so
    ACT chews ~20us of exp while the tensor engine is busy with GEMMs.
  - exp and the causal affine_select run e-batched over one [128, 2, w]
    AP covering both half-heads, halving ACT/GpSimd fixed overhead.
  - The per-(head,tj) normalize/combine epilogue is reworked to
    oh = U0 - lam'*U1 with lam' = lam*den0/den1 (RMS norm is
    scale-invariant; eps is kept exact via bias = eps*den0^2) and batched
    over all 8 tj of a head with stride-0 broadcast APs: ~10 DVE ops per
    head instead of ~40.
  - U accumulates in one [128,2,8,128] PSUM tile (4 banks); with the two
    S halves (4 banks) PSUM is exactly full.
  - q chunks scatter straight from PSUM into qTp (the q half of qkT was
    never read); input DMA issue is spread across sync/scalar queues and
    ordered so the first matmul's operands land first; qTp zero-fill is
    split across vector/gpsimd.
  - y is stored f16 (host sums partials in f32) on the scalar queue;
    outcat transposes go per-RMS-batch (heads 0-3 transpose while heads
    4-7 still compute).
"""

import math
from contextlib import ExitStack

import numpy as np

import concourse.bass as bass
import concourse.tile as tile
from concourse import mybir
from concourse.bass_utils import run_bass_kernel_spmd

# The deployed walrus rejects instructions carrying more than one sync wait
# ("Too many sync wait commands" in setupSyncWait).  Legalize at the BIR-JSON
# level: for every instruction with >1 wait, hoist the extra waits onto NoOp
# instructions inserted just before it on the same engine (engine streams are
# in-order, so semantics are identical).
_MAX_WAITS = 1


def _legalize_sync_waits(d):
    for f in d.get("functions", []):
        for bb in f.get("blocks", []):
            out = []
            for inst in bb["instructions"]:
                si = inst.get("sync_info")
                waits = (si or {}).get("on_wait") or []
                if len(waits) > _MAX_WAITS:
                    extra = waits[: len(waits) - _MAX_WAITS]
                    keep = waits[len(waits) - _MAX_WAITS :]
                    for j in range(0, len(extra), _MAX_WAITS):
                        nop = {
                            "engine": inst["engine"],
                            "ins": [],
                            "outs": [],
                            "name": f"{inst['name']}-lw{j}",
                            "opcode": "NoOp",
                            "sync_info": {
                                "on_wait": extra[j : j + _MAX_WAITS],
                                "on_update": [],
                            },
                        }
                        if "debug" in inst:
                            nop["debug"] = inst["debug"]
                        out.append(nop)
                    si["on_wait"] = keep
                out.append(inst)
            bb["instructions"] = out
    return d


_orig_to_json_bytes = bass.Bass.to_json_bytes


def _patched_to_json_bytes(self, *a, **kw):
    import json as _json

    raw = _orig_to_json_bytes(self, *a, **kw)
    return _json.dumps(_legalize_sync_waits(_json.loads(raw))).encode()


bass.Bass.to_json_bytes = _patched_to_json_bytes

F32 = mybir.dt.float32
F16 = mybir.dt.float16

B, T, C = 4, 1024, 1024
H_TOT = 16  # total v-heads
HD = 32  # half-head dim
DV = 64  # v-head dim
G = 2  # head groups (tensor parallel)
HPG = H_TOT // G  # 8 v-heads per core
COLS = 1024  # q cols + k cols per group
LAMBDA_INIT = 0.8 - 0.6 * math.exp(-0.3 * (1 - 1))  # 0.2
EPS = 1e-5
N_CORES = 8

NT = T // 128  # 8 t-tiles
NKC = C // 128  # 8 contraction chunks

# S/exp chunks per head: column ranges sized to fit a [128,2,512] PSUM
# half-buffer (per-e region stays inside one bank).
HEAD_CHUNKS = []
for _s in range(NT):
    _t0 = 128 * _s
    if _s < 4:
        HEAD_CHUNKS.append((_s, _t0, 512))
        HEAD_CHUNKS.append((_s, 512, 1024))
    else:
        HEAD_CHUNKS.append((_s, _t0, 1024))
NCH = len(HEAD_CHUNKS)  # 12


def _emit(ctx: ExitStack, tc: tile.TileContext, xT, w_qk, w_v, w_p, lam, y):
    nc = tc.nc
    AluOp = mybir.AluOpType
    Act = mybir.ActivationFunctionType

    const = ctx.enter_context(tc.tile_pool(name="const", bufs=1))
    lam_sb = const.tile([128, 1], F32)
    nc.sync.dma_start(out=lam_sb, in_=lam[:])

    big = ctx.enter_context(tc.tile_pool(name="big", bufs=1))
    kT_sb = big.tile([128, 4, T], F16)  # row-chunks of k^T [512, T]
    v_sb = big.tile([128, NT, HPG, 128], F16)  # [s-chunk][head][dv | ones]
    outcat_sb = big.tile([128, NT, HPG * DV], F16)  # [t-chunk][512]
    outcatT_sb = big.tile([128, 4, T], F16)  # row-chunks of [512, T]
    wp_sb = big.tile([128, 4, C], F16)
    # per-half-head q, zero-padded to K=128: data lives at the same 32-row
    # strip as that half-head's k rows inside its kT chunk, so the S^T
    # matmul can contract over the full 128 partitions at full stream rate
    # (the other half-heads' k rows meet zero q rows).
    qTp_sb = big.tile([128, 2 * HPG, T], F16)

    nc.vector.memset(qTp_sb[:, 0:8, :], 0.0)
    nc.gpsimd.memset(qTp_sb[:, 8:16, :], 0.0)
    nc.gpsimd.memset(v_sb[:, :, :, DV : DV + 1], 1.0)  # softmax-denominator col
    # ragged es tiles: tag per s-block, two generations deep
    es_pool = ctx.enter_context(tc.tile_pool(name="es", bufs=2))
    es_store = {}  # (h, s) -> tile [128, 2, T - 128*s]

    def exp_chunk(h, s, c0, c1, sbuf):
        """S matmuls for both e of chunk [c0:c1) into sbuf, fused exp into
        the ragged es tile, causal mask after the diagonal chunk."""
        t0 = 128 * s
        if c0 == t0:
            es_t = es_pool.tile([128, 2, T - t0], F16, tag=f"es{s}", name=f"es{h}_{s}")
            es_store[(h, s)] = es_t
        else:
            es_t = es_store[(h, s)]
        base = 0 if c0 < 512 else 512
        kc_ = h // 2
        for e in range(2):
            nc.tensor.matmul(
                sbuf[:, e, c0 - base : c1 - base],
                kT_sb[:, kc_, t0 : t0 + 128],
                qTp_sb[:, 2 * h + e, c0:c1],
                start=True,
                stop=True,
            )
        nc.scalar.activation(
            out=es_t[:, :, c0 - t0 : c1 - t0],
            in_=sbuf[:, :, c0 - base : c1 - base],
            func=Act.Exp,
            scale=1.0 / 32.0,
        )
        if c0 == t0:  # diagonal block: keep t >= s for both half-heads
            nc.gpsimd.affine_select(
                out=es_t[:, :, 0:128],
                in_=es_t[:, :, 0:128],
                pattern=[[0, 2], [1, 128]],
                compare_op=AluOp.is_ge,
                fill=0.0,
                base=0,
                channel_multiplier=-1,
            )

    # ---------------- phase 1+2: qkv projections ----------------
    with (
        tc.tile_pool(name="xw", bufs=1) as xw,
        tc.tile_pool(name="mmps", bufs=4, space="PSUM") as mmps,
    ):
        xT_sb = xw.tile([128, NKC, T], F16)
        wqk_sb = xw.tile([128, NKC, COLS], F16)
        wv_sb = xw.tile([128, NKC, 512], F16)

        xT_r = xT[:].rearrange("(c p) t -> p c t", p=128)
        wqk_r = w_qk[:].rearrange("(c p) m -> p c m", p=128)

        def load_wqk(cc):
            nc.sync.dma_start(
                out=wqk_sb[:, :, cc * 128 : (cc + 1) * 128],
                in_=wqk_r[:, :, cc * 128 : (cc + 1) * 128],
            )

        # order feeds the first matmuls first; xT/wv/wp issue on the scalar
        # queue so they don't serialize behind the wqk issues on sync.
        load_wqk(0)
        nc.scalar.dma_start(out=xT_sb[:, :, 0:512], in_=xT_r[:, :, 0:512])
        load_wqk(1)
        load_wqk(2)
        nc.scalar.dma_start(out=xT_sb[:, :, 512:1024], in_=xT_r[:, :, 512:1024])
        for cc in range(3, 8):
            load_wqk(cc)
        nc.scalar.dma_start(
            out=wv_sb, in_=w_v[:].rearrange("(c p) m -> p c m", p=128)
        )
        nc.scalar.dma_start(
            out=wp_sb, in_=w_p[:].rearrange("(c p) m -> p c m", p=128)
        )

        # chunk cc of [q^T; k^T] = w_qk[:, cc-block].T @ x^T
        for cc in range(8):
            for nh in range(2):
                ps = mmps.tile([128, 1024], F32, tag="mmps", name=f"qk{cc}{nh}")[:, 0:512]
                for kc in range(NKC):
                    nc.tensor.matmul(
                        ps,
                        wqk_sb[:, kc, cc * 128 : (cc + 1) * 128],
                        xT_sb[:, kc, nh * 512 : (nh + 1) * 512],
                        start=(kc == 0),
                        stop=(kc == NKC - 1),
                    )
                if cc < 4:  # q chunk: scatter the 4 half-heads into qTp
                    for j in range(4):
                        hh = cc * 4 + j
                        nc.vector.tensor_copy(
                            out=qTp_sb[
                                j * 32 : (j + 1) * 32,
                                hh,
                                nh * 512 : (nh + 1) * 512,
                            ],
                            in_=ps[j * 32 : (j + 1) * 32, :],
                        )
                else:  # k chunk
                    nc.vector.tensor_copy(
                        out=kT_sb[:, cc - 4, nh * 512 : (nh + 1) * 512], in_=ps
                    )

        # v projection interleaved with the full S/exp production for heads
        # 0-1 (24 chunks): ACT chews exp while the PE does the v GEMMs.
        pre_chunks = [(0, ch) for ch in HEAD_CHUNKS] + [(1, ch) for ch in HEAD_CHUNKS]
        pci = 0

        def emit_pre(upto):
            nonlocal pci
            while pci < min(upto, len(pre_chunks)):
                hp, (s, c0, c1) = pre_chunks[pci]
                sbuf = mmps.tile([128, 1024], F32, tag="mmps", name=f"pre{pci}")
                exp_chunk(hp, s, c0, c1, sbuf[:].rearrange("p (e w) -> p e w", e=2))
                pci += 1

        for tt in range(NT):
            ps = mmps.tile([128, 1024], F32, tag="mmps", name=f"v{tt}")[:, 0:512]
            for kc in range(NKC):
                nc.tensor.matmul(
                    ps,
                    xT_sb[:, kc, tt * 128 : (tt + 1) * 128],
                    wv_sb[:, kc, :],
                    start=(kc == 0),
                    stop=(kc == NKC - 1),
                )
            nc.vector.tensor_copy(
                out=v_sb[:, tt, :, 0:DV],
                in_=ps[:].rearrange("p (h d) -> p h d", h=HPG),
            )
            emit_pre(3 * (tt + 1))
        emit_pre(len(pre_chunks))

    # ---------------- phase 3: differential attention ----------------
    with (
        tc.tile_pool(name="sps", bufs=1, space="PSUM") as s_pool,
        tc.tile_pool(name="ups", bufs=1, space="PSUM") as u_pool,
        tc.tile_pool(name="comb", bufs=2) as comb,
        tc.tile_pool(name="ohp", bufs=1, space="SBUF") as ohp,
    ):
        oh_tiles = {}
        # per-(h,tj) rsqrt input: ssq/64 + eps*den0^2
        ssq_all = ohp.tile([128, HPG, NT], F32, tag="ssqall", name="ssqall")
        sb_idx = [0]

        def next_sbuf(name):
            t = s_pool.tile(
                [128, 2, 512], F32, tag=f"sh{sb_idx[0] % 2}", name=name
            )
            sb_idx[0] += 1
            return t

        def emit_av(h, s, u_big):
            # U[t-block, dv|den] += expS^T[s-block, t-block].T @ v_aug[s-block]
            es_t = es_store[(h, s)]
            for e in range(2):
                for tj in range(s, NT):
                    nc.tensor.matmul(
                        u_big[:, e, tj, 0 : DV + 1],
                        es_t[:, e, (tj - s) * 128 : (tj - s + 1) * 128],
                        v_sb[:, s, h, 0 : DV + 1],
                        start=(s == 0 and tj % 4 == 0),
                        stop=(s == tj and tj % 4 == 3),
                    )

        # ---- RMS in two batches: heads 0-3 overlap attention of heads 4-7
        def emit_rms(h_lo, h_hi):
            w = (h_hi - h_lo) * NT
            rstd = comb.tile([128, w], F32, tag=f"rstd{h_lo}", name=f"rstd{h_lo}")
            nc.scalar.activation(
                out=rstd, in_=ssq_all[:, h_lo:h_hi, :], func=Act.Sqrt
            )
            nc.vector.reciprocal(out=rstd, in_=rstd)
            for h in range(h_lo, h_hi):
                c0 = (h - h_lo) * NT
                nc.vector.tensor_mul(
                    outcat_sb[:, :, h * DV : (h + 1) * DV],
                    oh_tiles[h],
                    rstd[:, c0 : c0 + NT][:, :, None].broadcast_to([128, NT, DV]),
                )
            # heads 0-3 rows of outcat^T can transpose before heads 4-7 land
            lo2, hi2 = (0, 256) if h_hi < HPG else (256, 512)
            for tj in range(NT):
                nc.sync.dma_start_transpose(
                    out=outcatT_sb[:, lo2 // 128 : hi2 // 128, tj * 128 : (tj + 1) * 128],
                    in_=outcat_sb[:, tj, lo2:hi2],
                )

        for h in range(HPG):
            u_big = u_pool.tile([128, 2, NT, 128], F32, tag="ub", name=f"u_{h}")
            # producer for head h+2 rides along in this head's AV slot
            hp = h + 2
            np_ch = NCH if hp < HPG else 0
            ci = 0

            def emit_prod(upto):
                nonlocal ci
                while ci < min(upto, np_ch):
                    s, c0, c1 = HEAD_CHUNKS[ci]
                    exp_chunk(hp, s, c0, c1, next_sbuf(f"s{hp}_{ci}"))
                    ci += 1

            for s in range(NT):
                emit_prod((s + 1) * NCH // NT)
                emit_av(h, s, u_big)
            emit_prod(np_ch)

            # ---- batched normalize/combine epilogue --------------------
            # oh = U0/den0 - lam*U1/den1; RMS norm is scale-invariant, so
            # normalize(oh) == normalize(U0 - lam'*U1), lam' = lam*den0/den1,
            # with the eps term exact via bias = eps*den0^2:
            #   oh/rms(oh) = oh' * rsqrt(mean(oh'^2) + eps*den0^2)
            ub = u_big[:]  # [128, 2, NT, 128]
            U0 = ub[:, 0, :, 0:DV]
            U1 = ub[:, 1, :, 0:DV]
            d0 = ub[:, 0, :, DV : DV + 1]
            d1 = ub[:, 1, :, DV : DV + 1]
            rden = comb.tile([128, NT], F32, tag="rd", name=f"rd{h}")
            nc.vector.reciprocal(out=rden, in_=d1)
            # d0s = sqrt(eps)*den0 in SBUF (DVE ops may read only one PSUM
            # operand); lam_sb carries lam/sqrt(eps) so lamp = lam*den0/den1.
            d0s = comb.tile([128, NT], F32, tag="d0", name=f"d0{h}")
            nc.vector.tensor_scalar(
                out=d0s, in0=d0, scalar1=math.sqrt(EPS), scalar2=None, op0=AluOp.mult
            )
            lamp = comb.tile([128, NT], F32, tag="lp", name=f"lp{h}")
            nc.vector.scalar_tensor_tensor(
                out=lamp,
                in0=d0s,
                scalar=lam_sb[:],
                in1=rden,
                op0=AluOp.mult,
                op1=AluOp.mult,
            )
            tmp = comb.tile([128, NT, DV], F32, tag="tmp", name=f"tmp{h}")
            nc.vector.tensor_mul(
                tmp, U1, lamp[:, :, None].broadcast_to([128, NT, DV])
            )
            oh = ohp.tile([128, NT, DV], F32, tag=f"oh{h}", name=f"oh{h}")
            nc.vector.tensor_sub(oh, U0, tmp)
            sq = comb.tile([128, NT, DV], F32, tag="sq", name=f"sq{h}")
            nc.vector.tensor_mul(sq, oh, oh)
            nc.vector.tensor_reduce(
                out=ssq_all[:, h, :], in_=sq, axis=mybir.AxisListType.X, op=AluOp.add
            )
            bias = comb.tile([128, NT], F32, tag="bi", name=f"bi{h}")
            nc.vector.tensor_mul(bias, d0s, d0s)  # = eps*den0^2
            nc.vector.scalar_tensor_tensor(
                out=ssq_all[:, h, :],
                in0=ssq_all[:, h, :],
                scalar=1.0 / DV,
                in1=bias,
                op0=AluOp.mult,
                op1=AluOp.add,
            )
            oh_tiles[h] = oh
            if h == 3:
                emit_rms(0, 4)

        emit_rms(4, HPG)

    # ---------------- phase 4+5: output projection ----------------
    with (
        tc.tile_pool(name="pps", bufs=4, space="PSUM") as pps,
        tc.tile_pool(name="yout", bufs=2) as yout,
    ):
        for tt in range(NT):
            yt = yout.tile([128, C], F16, tag="yt", name=f"y{tt}")
            for nh in range(2):
                ps = pps.tile([128, 512], F32, tag="pp", name=f"pp{tt}{nh}")
                for rr in range(4):
                    nc.tensor.matmul(
                        ps,
                        outcatT_sb[:, rr, tt * 128 : (tt + 1) * 128],
                        wp_sb[:, rr, nh * 512 : (nh + 1) * 512],
                        start=(rr == 0),
                        stop=(rr == 3),
                    )
                nc.vector.tensor_copy(out=yt[:, nh * 512 : (nh + 1) * 512], in_=ps)
            nc.scalar.dma_start(out=y[tt * 128 : (tt + 1) * 128, :], in_=yt)


def build_nc():
    nc = bass.Bass()
    xT = nc.declare_dram_parameter("xT", [C, T], F16, isOutput=False)
    w_qk = nc.declare_dram_parameter("w_qk", [C, COLS], F16, isOutput=False)
    w_v = nc.declare_dram_parameter("w_v", [C, 512], F16, isOutput=False)
    w_p = nc.declare_dram_parameter("w_p", [512, C], F16, isOutput=False)
    lam = nc.declare_dram_parameter("lam", [128, 1], F32, isOutput=False)
    y = nc.declare_dram_parameter("y", [T, C], F16, isOutput=True)
    with tile.TileContext(nc) as tc:
        with ExitStack() as ctx:
            _emit(ctx, tc, xT, w_qk, w_v, w_p, lam, y)
    return nc


_NC = None


def _get_nc():
    global _NC
    if _NC is None:
        _NC = build_nc()
    return _NC


def make_in_maps(x, w_attn, w_proj, lambda_q1, lambda_q2, lambda_k1, lambda_k2, gamma):
    x = np.asarray(x, np.float32)
    w_attn = np.asarray(w_attn, np.float32)
    w_proj = np.asarray(w_proj, np.float32)
    lam1 = np.exp(np.sum(np.float32(lambda_q1) * np.float32(lambda_k1), dtype=np.float32))
    lam2 = np.exp(np.sum(np.float32(lambda_q2) * np.float32(lambda_k2), dtype=np.float32))
    lam_full = np.float32(lam1 - lam2 + LAMBDA_INIT)
    # kernel multiplies lam by d0s = sqrt(eps)*den0, so pre-divide here
    lam_tile = np.full((128, 1), lam_full / np.float32(math.sqrt(EPS)), np.float32)
    # fold gamma * (1 - lambda_init) into w_proj rows
    scale = np.tile(np.asarray(gamma, np.float32), H_TOT) * np.float32(1.0 - LAMBDA_INIT)
    w_p_full = (w_proj * scale[:, None]).astype(np.float16)

    in_maps = []
    for core in range(N_CORES):
        b, g = core // G, core % G
        in_maps.append(
            {
                "xT": np.ascontiguousarray(x[b].T.astype(np.float16)),
                "w_qk": np.ascontiguousarray(
                    np.concatenate(
                        [
                            w_attn[:, g * 512 : (g + 1) * 512],
                            w_attn[:, C + g * 512 : C + (g + 1) * 512],
                        ],
                        axis=1,
                    ).astype(np.float16)
                ),
                "w_v": np.ascontiguousarray(
                    w_attn[:, 2 * C + g * 512 : 2 * C + (g + 1) * 512].astype(
                        np.float16
                    )
                ),
                "w_p": np.ascontiguousarray(w_p_full[g * 512 : (g + 1) * 512, :]),
                "lam": lam_tile,
            }
        )
    return in_maps


def assemble(results):
    y = np.empty((B, T, C), np.float32)
    for b in range(B):
        y[b] = results[b * G]["y"].astype(np.float32) + results[b * G + 1][
            "y"
        ].astype(np.float32)
    return y


def kernel(**inputs) -> np.ndarray:
    nc = _get_nc()
    in_maps = make_in_maps(**inputs)
    res = run_bass_kernel_spmd(nc, in_maps, list(range(N_CORES)))
    return assemble(res.results)


# revision 16
# speedup vs baseline: 1.3889x; 1.0124x over previous
"""MultiHeadDiffAttn Trainium2 kernel (v4, lookahead producer/consumer).

Sharding: 8 cores = 4-way data parallel over batch x 2-way tensor parallel
over heads (8 v-heads / 16 half-heads per core).  Each core computes its
batch's qkv projection restricted to its head group, differential attention
with per-half-head softmax, head RMS norm, and a partial output projection
(its 512 rows of w_proj).  Host sums the two partial projections per batch.

Structure (what the measured traces drove):
  - ACT (exp) is the attention-phase bottleneck while the qkv phase leaves
    it idle, and a single S buffer serializes S(s+1) behind exp(s) (WAR).
    So S/exp/mask production is decoupled from AV consumption: S goes
    through two half-size PSUM buffers (2x [128,2,512], chunks alternate),
    exp writes ragged es tiles (only cols [t0:T]), and the producer for
    head h+2 is interleaved into head h's AV slot.  Heads 0-1 produce
    entirely inside the qkv phase (interleaved with the v-projection), so
    ACT chews ~20us of exp while the tensor engine is busy with GEMMs.
  - exp and the causal affine_select run e-batched over one [128, 2, w]
    AP covering both half-heads, halving ACT/GpSimd fixed overhead.
  - The per-(head,tj) normalize/combine epilogue is reworked to
    oh = U0 - lam'*U1 with lam' = lam*den0/den1 (RMS norm is
    scale-invariant; eps is kept exact via bias = eps*den0^2) and batched
    over all 8 tj of a head with stride-0 broadcast APs: ~10 DVE ops per
    head instead of ~40.
  - U accumulates in one [128,2,8,128] PSUM tile (4 banks); with the two
    S halves (4 banks) PSUM is exactly full.
  - q chunks scatter straight from PSUM into qTp (the q half of qkT was
    never read); input DMA issue is spread across sync/scalar queues and
    ordered so the first matmul's operands land first; qTp zero-fill is
    split across vector/gpsimd.
  - y is stored f16 (host sums partials in f32) on the scalar queue;
    outcat transposes go per-RMS-batch (heads 0-3 transpose while heads
    4-7 still compute).
"""

import math
from contextlib import ExitStack

import numpy as np

import concourse.bass as bass
import concourse.tile as tile
from concourse import mybir
from concourse.bass_utils import run_bass_kernel_spmd

# The deployed walrus rejects instructions carrying more than one sync wait
# ("Too many sync wait commands" in setupSyncWait).  Legalize at the BIR-JSON
# level: for every instruction with >1 wait, hoist the extra waits onto NoOp
# instructions inserted just before it on the same engine (engine streams are
# in-order, so semantics are identical).
_MAX_WAITS = 1


def _legalize_sync_waits(d):
    for f in d.get("functions", []):
        for bb in f.get("blocks", []):
            out = []
            for inst in bb["instructions"]:
                si = inst.get("sync_info")
                waits = (si or {}).get("on_wait") or []
                if len(waits) > _MAX_WAITS:
                    extra = waits[: len(waits) - _MAX_WAITS]
                    keep = waits[len(waits) - _MAX_WAITS :]
                    for j in range(0, len(extra), _MAX_WAITS):
                        nop = {
                            "engine": inst["engine"],
                            "ins": [],
                            "outs": [],
                            "name": f"{inst['name']}-lw{j}",
                            "opcode": "NoOp",
                            "sync_info": {
                                "on_wait": extra[j : j + _MAX_WAITS],
                                "on_update": [],
                            },
                        }
                        if "debug" in inst:
                            nop["debug"] = inst["debug"]
                        out.append(nop)
                    si["on_wait"] = keep
                out.append(inst)
            bb["instructions"] = out
    return d


_orig_to_json_bytes = bass.Bass.to_json_bytes


def _patched_to_json_bytes(self, *a, **kw):
    import json as _json

    raw = _orig_to_json_bytes(self, *a, **kw)
    return _json.dumps(_legalize_sync_waits(_json.loads(raw))).encode()


bass.Bass.to_json_bytes = _patched_to_json_bytes

F32 = mybir.dt.float32
F16 = mybir.dt.float16

B, T, C = 4, 1024, 1024
H_TOT = 16  # total v-heads
HD = 32  # half-head dim
DV = 64  # v-head dim
G = 2  # head groups (tensor parallel)
HPG = H_TOT // G  # 8 v-heads per core
COLS = 1024  # q cols + k cols per group
LAMBDA_INIT = 0.8 - 0.6 * math.exp(-0.3 * (1 - 1))  # 0.2
EPS = 1e-5
N_CORES = 8

NT = T // 128  # 8 t-tiles
NKC = C // 128  # 8 contraction chunks

# S/exp chunks per head: column ranges sized to fit a [128,2,512] PSUM
# half-buffer (per-e region stays inside one bank).
HEAD_CHUNKS = []
for _s in range(NT):
    _t0 = 128 * _s
    if _s < 4:
        HEAD_CHUNKS.append((_s, _t0, 512))
        HEAD_CHUNKS.append((_s, 512, 1024))
    else:
        HEAD_CHUNKS.append((_s, _t0, 1024))
NCH = len(HEAD_CHUNKS)  # 12


def _emit(ctx: ExitStack, tc: tile.TileContext, xT, w_qk, w_v, w_p, lam, y):
    nc = tc.nc
    AluOp = mybir.AluOpType
    Act = mybir.ActivationFunctionType

    const = ctx.enter_context(tc.tile_pool(name="const", bufs=1))
    lam_sb = const.tile([128, 1], F32)
    nc.sync.dma_start(out=lam_sb, in_=lam[:])

    big = ctx.enter_context(tc.tile_pool(name="big", bufs=1))
    kT_sb = big.tile([128, 4, T], F16)  # row-chunks of k^T [512, T]
    v_sb = big.tile([128, NT, HPG, 128], F16)  # [s-chunk][head][dv | ones]
    outcat_sb = big.tile([128, NT, HPG * DV], F16)  # [t-chunk][512]
    outcatT_sb = big.tile([128, 4, T], F16)  # row-chunks of [512, T]
    wp_sb = big.tile([128, 4, C], F16)
    # per-half-head q, zero-padded to K=128: data lives at the same 32-row
    # strip as that half-head's k rows inside its kT chunk, so the S^T
    # matmul can contract over the full 128 partitions at full stream rate
    # (the other half-heads' k rows meet zero q rows).
    qTp_sb = big.tile([128, 2 * HPG, T], F16)

    nc.vector.memset(qTp_sb[:, 0:8, :], 0.0)
    nc.gpsimd.memset(qTp_sb[:, 8:16, :], 0.0)
    nc.gpsimd.memset(v_sb[:, :, :, DV : DV + 1], 1.0)  # softmax-denominator col
    # ragged es tiles: tag per s-block, two generations deep
    es_pool = ctx.enter_context(tc.tile_pool(name="es", bufs=2))
    es_store = {}  # (h, s) -> tile [128, 2, T - 128*s]

    def exp_chunk(h, s, c0, c1, sbuf):
        """S matmuls for both e of chunk [c0:c1) into sbuf, fused exp into
        the ragged es tile, causal mask after the diagonal chunk."""
        t0 = 128 * s
        if c0 == t0:
            es_t = es_pool.tile([128, 2, T - t0], F16, tag=f"es{s}", name=f"es{h}_{s}")
            es_store[(h, s)] = es_t
        else:
            es_t = es_store[(h, s)]
        base = 0 if c0 < 512 else 512
        kc_ = h // 2
        for e in range(2):
            nc.tensor.matmul(
                sbuf[:, e, c0 - base : c1 - base],
                kT_sb[:, kc_, t0 : t0 + 128],
                qTp_sb[:, 2 * h + e, c0:c1],
                start=True,
                stop=True,
            )
        nc.scalar.activation(
            out=es_t[:, :, c0 - t0 : c1 - t0],
            in_=sbuf[:, :, c0 - base : c1 - base],
            func=Act.Exp,
            scale=1.0 / 32.0,
        )
        if c0 == t0:  # diagonal block: keep t >= s for both half-heads
            nc.gpsimd.affine_select(
                out=es_t[:, :, 0:128],
                in_=es_t[:, :, 0:128],
                pattern=[[0, 2], [1, 128]],
                compare_op=AluOp.is_ge,
                fill=0.0,
                base=0,
                channel_multiplier=-1,
            )

    # ---------------- phase 1+2: qkv projections ----------------
    with (
        tc.tile_pool(name="xw", bufs=1) as xw,
        tc.tile_pool(name="mmps", bufs=4, space="PSUM") as mmps,
    ):
        # host pre-transposes everything into SBUF-ready, per-partition
        # contiguous layouts so each DMA is a handful of big descriptors
        # instead of ~1k strided 256B packets.
        xT_sb = xw.tile([128, 2, NKC, 512], F16)  # [nh][kc][cols]
        wqk_sb = xw.tile([128, 8, NKC, 128], F16)  # [cc][kc][cols]
        wv_sb = xw.tile([128, NKC, 512], F16)

        def load_wqk(cc):
            nc.sync.dma_start(out=wqk_sb[:, cc, :, :], in_=w_qk[:, cc, :, :])

        # order feeds the first matmuls first; xT/wv/wp issue on the scalar
        # queue so they don't serialize behind the wqk issues on sync.
        load_wqk(0)
        nc.scalar.dma_start(out=xT_sb[:, 0, :, :], in_=xT[:, 0, :, :])
        load_wqk(1)
        load_wqk(2)
        nc.scalar.dma_start(out=xT_sb[:, 1, :, :], in_=xT[:, 1, :, :])
        for cc in range(3, 8):
            load_wqk(cc)
        nc.scalar.dma_start(out=wv_sb, in_=w_v[:])
        nc.scalar.dma_start(out=wp_sb, in_=w_p[:])

        # chunk cc of [q^T; k^T] = w_qk[:, cc-block].T @ x^T
        for cc in range(8):
            for nh in range(2):
                ps = mmps.tile([128, 1024], F32, tag="mmps", name=f"qk{cc}{nh}")[:, 0:512]
                for kc in range(NKC):
                    nc.tensor.matmul(
                        ps,
                        wqk_sb[:, cc, kc, :],
                        xT_sb[:, nh, kc, :],
                        start=(kc == 0),
                        stop=(kc == NKC - 1),
                    )
                if cc < 4:  # q chunk: scatter the 4 half-heads into qTp
                    # split between DVE and the (idle-until-prebake) ACT
                    for j in range(4):
                        hh = cc * 4 + j
                        dst = qTp_sb[
                            j * 32 : (j + 1) * 32, hh, nh * 512 : (nh + 1) * 512
                        ]
                        src = ps[j * 32 : (j + 1) * 32, :]
                        if j < 2:
                            nc.vector.tensor_copy(out=dst, in_=src)
                        else:
                            nc.scalar.copy(out=dst, in_=src)
                else:  # k chunk
                    nc.vector.tensor_copy(
                        out=kT_sb[:, cc - 4, nh * 512 : (nh + 1) * 512], in_=ps
                    )

        # v projection interleaved with the full S/exp production for heads
        # 0-1 (24 chunks): ACT chews exp while the PE does the v GEMMs.
        pre_chunks = [(0, ch) for ch in HEAD_CHUNKS] + [(1, ch) for ch in HEAD_CHUNKS]
        pci = 0

        def emit_pre(upto):
            nonlocal pci
            while pci < min(upto, len(pre_chunks)):
                hp, (s, c0, c1) = pre_chunks[pci]
                sbuf = mmps.tile([128, 1024], F32, tag="mmps", name=f"pre{pci}")
                exp_chunk(hp, s, c0, c1, sbuf[:].rearrange("p (e w) -> p e w", e=2))
                pci += 1

        for tt in range(NT):
            ps = mmps.tile([128, 1024], F32, tag="mmps", name=f"v{tt}")[:, 0:512]
            for kc in range(NKC):
                nc.tensor.matmul(
                    ps,
                    xT_sb[:, tt // 4, kc, (tt % 4) * 128 : (tt % 4 + 1) * 128],
                    wv_sb[:, kc, :],
                    start=(kc == 0),
                    stop=(kc == NKC - 1),
                )
            nc.vector.tensor_copy(
                out=v_sb[:, tt, :, 0:DV],
                in_=ps[:].rearrange("p (h d) -> p h d", h=HPG),
            )
            emit_pre(3 * (tt + 1))
        emit_pre(len(pre_chunks))

    # ---------------- phase 3: differential attention ----------------
    with (
        tc.tile_pool(name="sps", bufs=1, space="PSUM") as s_pool,
        tc.tile_pool(name="ups", bufs=1, space="PSUM") as u_pool,
        tc.tile_pool(name="comb", bufs=2) as comb,
        tc.tile_pool(name="ohp", bufs=1, space="SBUF") as ohp,
    ):
        oh_tiles = {}
        # per-(h,tj) rsqrt input: ssq/64 + eps*den0^2
        ssq_all = ohp.tile([128, HPG, NT], F32, tag="ssqall", name="ssqall")
        sb_idx = [0]

        def next_sbuf(name):
            t = s_pool.tile(
                [128, 2, 512], F32, tag=f"sh{sb_idx[0] % 2}", name=name
            )
            sb_idx[0] += 1
            return t

        def emit_av(h, s, u_big):
            # U[t-block, dv|den] += expS^T[s-block, t-block].T @ v_aug[s-block]
            es_t = es_store[(h, s)]
            for e in range(2):
                for tj in range(s, NT):
                    nc.tensor.matmul(
                        u_big[:, e, tj, 0 : DV + 1],
                        es_t[:, e, (tj - s) * 128 : (tj - s + 1) * 128],
                        v_sb[:, s, h, 0 : DV + 1],
                        start=(s == 0 and tj % 4 == 0),
                        stop=(s == tj and tj % 4 == 3),
                    )

        # ---- RMS in sub-batches so outcat rows transpose while later heads
        # still compute; apply-muls go on gpsimd to keep DVE free for the
        # next head's epilogue (which gates its AV matmuls via u_big).
        def emit_rms(h_lo, h_hi, dma_eng):
            w = (h_hi - h_lo) * NT
            rstd = comb.tile([128, w], F32, tag=f"rstd{h_lo}", name=f"rstd{h_lo}")
            nc.scalar.activation(
                out=rstd, in_=ssq_all[:, h_lo:h_hi, :], func=Act.Sqrt
            )
            nc.vector.reciprocal(out=rstd, in_=rstd)
            for h in range(h_lo, h_hi):
                c0 = (h - h_lo) * NT
                nc.gpsimd.tensor_mul(
                    outcat_sb[:, :, h * DV : (h + 1) * DV],
                    oh_tiles[h],
                    rstd[:, c0 : c0 + NT][:, :, None].broadcast_to([128, NT, DV]),
                )
            lo2, hi2 = h_lo * DV, h_hi * DV
            for tj in range(NT):
                dma_eng.dma_start_transpose(
                    out=outcatT_sb[:, lo2 // 128 : hi2 // 128, tj * 128 : (tj + 1) * 128],
                    in_=outcat_sb[:, tj, lo2:hi2],
                )

        for h in range(HPG):
            u_big = u_pool.tile([128, 2, NT, 128], F32, tag="ub", name=f"u_{h}")
            # producer for head h+2 rides along in this head's AV slot
            hp = h + 2
            np_ch = NCH if hp < HPG else 0
            ci = 0

            def emit_prod(upto):
                nonlocal ci
                while ci < min(upto, np_ch):
                    s, c0, c1 = HEAD_CHUNKS[ci]
                    exp_chunk(hp, s, c0, c1, next_sbuf(f"s{hp}_{ci}"))
                    ci += 1

            for s in range(NT):
                emit_prod((s + 1) * NCH // NT)
                emit_av(h, s, u_big)
            emit_prod(np_ch)

            # ---- batched normalize/combine epilogue --------------------
            # oh = U0/den0 - lam*U1/den1; RMS norm is scale-invariant, so
            # normalize(oh) == normalize(U0 - lam'*U1), lam' = lam*den0/den1,
            # with the eps term exact via bias = eps*den0^2:
            #   oh/rms(oh) = oh' * rsqrt(mean(oh'^2) + eps*den0^2)
            ub = u_big[:]  # [128, 2, NT, 128]
            U0 = ub[:, 0, :, 0:DV]
            U1 = ub[:, 1, :, 0:DV]
            d0 = ub[:, 0, :, DV : DV + 1]
            d1 = ub[:, 1, :, DV : DV + 1]
            rden = comb.tile([128, NT], F32, tag="rd", name=f"rd{h}")
            nc.vector.reciprocal(out=rden, in_=d1)
            # d0s = sqrt(eps)*den0 in SBUF (DVE ops may read only one PSUM
            # operand); lam_sb carries lam/sqrt(eps) so lamp = lam*den0/den1.
            d0s = comb.tile([128, NT], F32, tag="d0", name=f"d0{h}")
            nc.vector.tensor_scalar(
                out=d0s, in0=d0, scalar1=math.sqrt(EPS), scalar2=None, op0=AluOp.mult
            )
            lamp = comb.tile([128, NT], F32, tag="lp", name=f"lp{h}")
            nc.vector.scalar_tensor_tensor(
                out=lamp,
                in0=d0s,
                scalar=lam_sb[:],
                in1=rden,
                op0=AluOp.mult,
                op1=AluOp.mult,
            )
            tmp = comb.tile([128, NT, DV], F32, tag="tmp", name=f"tmp{h}")
            nc.vector.tensor_mul(
                tmp, U1, lamp[:, :, None].broadcast_to([128, NT, DV])
            )
            oh = ohp.tile([128, NT, DV], F32, tag=f"oh{h}", name=f"oh{h}")
            nc.vector.tensor_sub(oh, U0, tmp)
            sq = comb.tile([128, NT, DV], F32, tag="sq", name=f"sq{h}")
            nc.vector.tensor_mul(sq, oh, oh)
            nc.vector.tensor_reduce(
                out=ssq_all[:, h, :], in_=sq, axis=mybir.AxisListType.X, op=AluOp.add
            )
            bias = comb.tile([128, NT], F32, tag="bi", name=f"bi{h}")
            nc.vector.tensor_mul(bias, d0s, d0s)  # = eps*den0^2
            nc.vector.scalar_tensor_tensor(
                out=ssq_all[:, h, :],
                in0=ssq_all[:, h, :],
                scalar=1.0 / DV,
                in1=bias,
                op0=AluOp.mult,
                op1=AluOp.add,
            )
            oh_tiles[h] = oh
            if h == 3:
                emit_rms(0, 4, nc.sync)
            elif h == 5:
                emit_rms(4, 6, nc.sync)

        emit_rms(6, HPG, nc.scalar)

    # ---------------- phase 4+5: output projection ----------------
    with (
        tc.tile_pool(name="pps", bufs=4, space="PSUM") as pps,
        tc.tile_pool(name="yout", bufs=2) as yout,
    ):
        for tt in range(NT):
            yt = yout.tile([128, C], F16, tag="yt", name=f"y{tt}")
            for nh in range(2):
                ps = pps.tile([128, 512], F32, tag="pp", name=f"pp{tt}{nh}")
                for rr in range(4):
                    nc.tensor.matmul(
                        ps,
                        outcatT_sb[:, rr, tt * 128 : (tt + 1) * 128],
                        wp_sb[:, rr, nh * 512 : (nh + 1) * 512],
                        start=(rr == 0),
                        stop=(rr == 3),
                    )
                nc.vector.tensor_copy(out=yt[:, nh * 512 : (nh + 1) * 512], in_=ps)
            nc.sync.dma_start(out=y[tt * 128 : (tt + 1) * 128, :], in_=yt)


def build_nc():
    nc = bass.Bass()
    # host pre-transposed, per-partition contiguous layouts (see make_in_maps)
    xT = nc.declare_dram_parameter("xT", [128, 2, NKC, 512], F16, isOutput=False)
    w_qk = nc.declare_dram_parameter("w_qk", [128, 8, NKC, 128], F16, isOutput=False)
    w_v = nc.declare_dram_parameter("w_v", [128, NKC, 512], F16, isOutput=False)
    w_p = nc.declare_dram_parameter("w_p", [128, 4, C], F16, isOutput=False)
    lam = nc.declare_dram_parameter("lam", [128, 1], F32, isOutput=False)
    y = nc.declare_dram_parameter("y", [T, C], F16, isOutput=True)
    with tile.TileContext(nc) as tc:
        with ExitStack() as ctx:
            _emit(ctx, tc, xT, w_qk, w_v, w_p, lam, y)
    return nc


_NC = None


def _get_nc():
    global _NC
    if _NC is None:
        _NC = build_nc()
    return _NC


def make_in_maps(x, w_attn, w_proj, lambda_q1, lambda_q2, lambda_k1, lambda_k2, gamma):
    x = np.asarray(x, np.float32)
    w_attn = np.asarray(w_attn, np.float32)
    w_proj = np.asarray(w_proj, np.float32)
    lam1 = np.exp(np.sum(np.float32(lambda_q1) * np.float32(lambda_k1), dtype=np.float32))
    lam2 = np.exp(np.sum(np.float32(lambda_q2) * np.float32(lambda_k2), dtype=np.float32))
    lam_full = np.float32(lam1 - lam2 + LAMBDA_INIT)
    # kernel multiplies lam by d0s = sqrt(eps)*den0, so pre-divide here
    lam_tile = np.full((128, 1), lam_full / np.float32(math.sqrt(EPS)), np.float32)
    # fold gamma * (1 - lambda_init) into w_proj rows
    scale = np.tile(np.asarray(gamma, np.float32), H_TOT) * np.float32(1.0 - LAMBDA_INIT)
    w_p_full = (w_proj * scale[:, None]).astype(np.float16)

    in_maps = []
    for core in range(N_CORES):
        b, g = core // G, core % G
        # per-partition contiguous DRAM layouts matching the SBUF tiles
        xt = x[b].T.astype(np.float16)  # [C, T]
        xt4 = xt.reshape(NKC, 128, 2, 512).transpose(1, 2, 0, 3)
        wqk = np.concatenate(
            [
                w_attn[:, g * 512 : (g + 1) * 512],
                w_attn[:, C + g * 512 : C + (g + 1) * 512],
            ],
            axis=1,
        ).astype(np.float16)  # [C, COLS]
        wqk4 = wqk.reshape(NKC, 128, 8, 128).transpose(1, 2, 0, 3)
        wv = w_attn[:, 2 * C + g * 512 : 2 * C + (g + 1) * 512].astype(np.float16)
        wv3 = wv.reshape(NKC, 128, 512).transpose(1, 0, 2)
        wp3 = w_p_full[g * 512 : (g + 1) * 512, :].reshape(4, 128, C).transpose(1, 0, 2)
        in_maps.append(
            {
                "xT": np.ascontiguousarray(xt4),
                "w_qk": np.ascontiguousarray(wqk4),
                "w_v": np.ascontiguousarray(wv3),
                "w_p": np.ascontiguousarray(wp3),
                "lam": lam_tile,
            }
        )
    return in_maps


def assemble(results):
    y = np.empty((B, T, C), np.float32)
    for b in range(B):
        y[b] = results[b * G]["y"].astype(np.float32) + results[b * G + 1][
            "y"
        ].astype(np.float32)
    return y


def kernel(**inputs) -> np.ndarray:
    nc = _get_nc()
    in_maps = make_in_maps(**inputs)
    res = run_bass_kernel_spmd(nc, in_maps, list(range(N_CORES)))
    return assemble(res.results)


# revision 21
# speedup vs baseline: 1.4365x; 1.0343x over previous
"""MultiHeadDiffAttn Trainium2 kernel (v4, lookahead producer/consumer).

Sharding: 8 cores = 4-way data parallel over batch x 2-way tensor parallel
over heads (8 v-heads / 16 half-heads per core).  Each core computes its
batch's qkv projection restricted to its head group, differential attention
with per-half-head softmax, head RMS norm, and a partial output projection
(its 512 rows of w_proj).  Host sums the two partial projections per batch.

Structure (what the measured traces drove):
  - ACT (exp) is the attention-phase bottleneck while the qkv phase leaves
    it idle, and a single S buffer serializes S(s+1) behind exp(s) (WAR).
    So S/exp/mask production is decoupled from AV consumption: S goes
    through two half-size PSUM buffers (2x [128,2,512], chunks alternate),
    exp writes ragged es tiles (only cols [t0:T]), and the producer for
    head h+2 is interleaved into head h's AV slot.  Heads 0-1 produce
    entirely inside the qkv phase (interleaved with the v-projection), so
    ACT chews ~20us of exp while the tensor engine is busy with GEMMs.
  - exp and the causal affine_select run e-batched over one [128, 2, w]
    AP covering both half-heads, halving ACT/GpSimd fixed overhead.
  - The per-(head,tj) normalize/combine epilogue is reworked to
    oh = U0 - lam'*U1 with lam' = lam*den0/den1 (RMS norm is
    scale-invariant; eps is kept exact via bias = eps*den0^2) and batched
    over all 8 tj of a head with stride-0 broadcast APs: ~10 DVE ops per
    head instead of ~40.
  - U accumulates in one [128,2,8,128] PSUM tile (4 banks); with the two
    S halves (4 banks) PSUM is exactly full.
  - q chunks scatter straight from PSUM into qTp (the q half of qkT was
    never read); input DMA issue is spread across sync/scalar queues and
    ordered so the first matmul's operands land first; qTp zero-fill is
    split across vector/gpsimd.
  - y is stored f16 (host sums partials in f32) on the scalar queue;
    outcat transposes go per-RMS-batch (heads 0-3 transpose while heads
    4-7 still compute).
"""

import math
from contextlib import ExitStack

import numpy as np

import concourse.bass as bass
import concourse.tile as tile
from concourse import mybir
from concourse.bass_utils import run_bass_kernel_spmd

# The deployed walrus rejects instructions carrying more than one sync wait
# ("Too many sync wait commands" in setupSyncWait).  Legalize at the BIR-JSON
# level: for every instruction with >1 wait, hoist the extra waits onto NoOp
# instructions inserted just before it on the same engine (engine streams are
# in-order, so semantics are identical).
_MAX_WAITS = 1


def _legalize_sync_waits(d):
    for f in d.get("functions", []):
        for bb in f.get("blocks", []):
            out = []
            for inst in bb["instructions"]:
                si = inst.get("sync_info")
                waits = (si or {}).get("on_wait") or []
                if len(waits) > _MAX_WAITS:
                    extra = waits[: len(waits) - _MAX_WAITS]
                    keep = waits[len(waits) - _MAX_WAITS :]
                    for j in range(0, len(extra), _MAX_WAITS):
                        nop = {
                            "engine": inst["engine"],
                            "ins": [],
                            "outs": [],
                            "name": f"{inst['name']}-lw{j}",
                            "opcode": "NoOp",
                            "sync_info": {
                                "on_wait": extra[j : j + _MAX_WAITS],
                                "on_update": [],
                            },
                        }
                        if "debug" in inst:
                            nop["debug"] = inst["debug"]
                        out.append(nop)
                    si["on_wait"] = keep
                out.append(inst)
            bb["instructions"] = out
    return d


_orig_to_json_bytes = bass.Bass.to_json_bytes


def _patched_to_json_bytes(self, *a, **kw):
    import json as _json

    raw = _orig_to_json_bytes(self, *a, **kw)
    return _json.dumps(_legalize_sync_waits(_json.loads(raw))).encode()


bass.Bass.to_json_bytes = _patched_to_json_bytes

F32 = mybir.dt.float32
F16 = mybir.dt.float16

B, T, C = 4, 1024, 1024
H_TOT = 16  # total v-heads
HD = 32  # half-head dim
DV = 64  # v-head dim
G = 2  # head groups (tensor parallel)
HPG = H_TOT // G  # 8 v-heads per core
COLS = 1024  # q cols + k cols per group
LAMBDA_INIT = 0.8 - 0.6 * math.exp(-0.3 * (1 - 1))  # 0.2
EPS = 1e-5
N_CORES = 8

NT = T // 128  # 8 t-tiles
NKC = C // 128  # 8 contraction chunks

# S/exp chunks per head: column ranges sized to fit a [128,2,512] PSUM
# half-buffer (per-e region stays inside one bank).
HEAD_CHUNKS = []
for _s in range(NT):
    _t0 = 128 * _s
    if _s < 4:
        HEAD_CHUNKS.append((_s, _t0, 512))
        HEAD_CHUNKS.append((_s, 512, 1024))
    else:
        HEAD_CHUNKS.append((_s, _t0, 1024))
NCH = len(HEAD_CHUNKS)  # 12


def _emit(ctx: ExitStack, tc: tile.TileContext, xT, w_qk, w_v, w_p, lam, y):
    nc = tc.nc
    AluOp = mybir.AluOpType
    Act = mybir.ActivationFunctionType

    const = ctx.enter_context(tc.tile_pool(name="const", bufs=1))
    lam_sb = const.tile([128, 1], F32)
    nc.sync.dma_start(out=lam_sb, in_=lam[:])

    big = ctx.enter_context(tc.tile_pool(name="big", bufs=1))
    kT_sb = big.tile([128, 4, T], F16)  # row-chunks of k^T [512, T]
    v_sb = big.tile([128, NT, HPG, 128], F16)  # [s-chunk][head][dv | ones]
    # [head-pair chunk][t-chunk][128 ch]: per-chunk contiguous so one DMA
    # transpose handles all 8 t-chunks of a head pair
    outcat_sb = big.tile([128, 4, NT, 128], F16)
    outcatT_sb = big.tile([128, 4, T], F16)  # row-chunks of [512, T]
    wp_sb = big.tile([128, 4, C], F16)
    # per-half-head q, zero-padded to K=128: data lives at the same 32-row
    # strip as that half-head's k rows inside its kT chunk, so the S^T
    # matmul can contract over the full 128 partitions at full stream rate
    # (the other half-heads' k rows meet zero q rows).
    qTp_sb = big.tile([128, 2 * HPG, T], F16)

    nc.vector.memset(qTp_sb[:, 0:8, :], 0.0)
    nc.gpsimd.memset(qTp_sb[:, 8:16, :], 0.0)
    nc.gpsimd.memset(v_sb[:, :, :, DV : DV + 1], 1.0)  # softmax-denominator col
    # ragged es tiles: tag per s-block, two generations deep
    es_pool = ctx.enter_context(tc.tile_pool(name="es", bufs=2))
    es_store = {}  # (h, s) -> tile [128, 2, T - 128*s]

    def exp_chunk(h, s, c0, c1, sbuf):
        """S matmuls for both e of chunk [c0:c1) into sbuf, fused exp into
        the ragged es tile, causal mask after the diagonal chunk."""
        t0 = 128 * s
        if c0 == t0:
            es_t = es_pool.tile([128, 2, T - t0], F16, tag=f"es{s}", name=f"es{h}_{s}")
            es_store[(h, s)] = es_t
        else:
            es_t = es_store[(h, s)]
        base = 0 if c0 < 512 else 512
        kc_ = h // 2
        for e in range(2):
            nc.tensor.matmul(
                sbuf[:, e, c0 - base : c1 - base],
                kT_sb[:, kc_, t0 : t0 + 128],
                qTp_sb[:, 2 * h + e, c0:c1],
                start=True,
                stop=True,
            )
        nc.scalar.activation(
            out=es_t[:, :, c0 - t0 : c1 - t0],
            in_=sbuf[:, :, c0 - base : c1 - base],
            func=Act.Exp,
            scale=1.0 / 32.0,
        )
        if c0 == t0:  # diagonal block: keep t >= s for both half-heads
            nc.gpsimd.affine_select(
                out=es_t[:, :, 0:128],
                in_=es_t[:, :, 0:128],
                pattern=[[0, 2], [1, 128]],
                compare_op=AluOp.is_ge,
                fill=0.0,
                base=0,
                channel_multiplier=-1,
            )

    # ---------------- phase 1+2: qkv projections ----------------
    with (
        tc.tile_pool(name="xw", bufs=1) as xw,
        tc.tile_pool(name="mmps", bufs=4, space="PSUM") as mmps,
    ):
        # host pre-transposes everything into SBUF-ready, per-partition
        # contiguous layouts so each DMA is a handful of big descriptors
        # instead of ~1k strided 256B packets.
        xT_sb = xw.tile([128, 2, NKC, 512], F16)  # [nh][kc][cols]
        wqk_sb = xw.tile([128, 8, NKC, 128], F16)  # [cc][kc][cols]
        wv_sb = xw.tile([128, NKC, 512], F16)

        def load_wqk(cc):
            nc.sync.dma_start(out=wqk_sb[:, cc, :, :], in_=w_qk[:, cc, :, :])

        # order feeds the first matmuls first; xT/wv/wp issue on the scalar
        # queue so they don't serialize behind the wqk issues on sync.
        load_wqk(0)
        nc.scalar.dma_start(out=xT_sb[:, 0, :, :], in_=xT[:, 0, :, :])
        load_wqk(1)
        load_wqk(2)
        nc.scalar.dma_start(out=xT_sb[:, 1, :, :], in_=xT[:, 1, :, :])
        for cc in range(3, 8):
            load_wqk(cc)
        nc.scalar.dma_start(out=wv_sb, in_=w_v[:])
        nc.scalar.dma_start(out=wp_sb, in_=w_p[:])

        # chunk cc of [q^T; k^T] = w_qk[:, cc-block].T @ x^T
        for cc in range(8):
            for nh in range(2):
                ps = mmps.tile([128, 1024], F32, tag="mmps", name=f"qk{cc}{nh}")[:, 0:512]
                for kc in range(NKC):
                    nc.tensor.matmul(
                        ps,
                        wqk_sb[:, cc, kc, :],
                        xT_sb[:, nh, kc, :],
                        start=(kc == 0),
                        stop=(kc == NKC - 1),
                    )
                if cc < 4:  # q chunk: scatter the 4 half-heads into qTp
                    # split between DVE and the (idle-until-prebake) ACT
                    for j in range(4):
                        hh = cc * 4 + j
                        dst = qTp_sb[
                            j * 32 : (j + 1) * 32, hh, nh * 512 : (nh + 1) * 512
                        ]
                        src = ps[j * 32 : (j + 1) * 32, :]
                        if j < 2:
                            nc.vector.tensor_copy(out=dst, in_=src)
                        else:
                            nc.scalar.copy(out=dst, in_=src)
                else:  # k chunk
                    nc.vector.tensor_copy(
                        out=kT_sb[:, cc - 4, nh * 512 : (nh + 1) * 512], in_=ps
                    )

        # v projection interleaved with the full S/exp production for heads
        # 0-1 (24 chunks): ACT chews exp while the PE does the v GEMMs.
        pre_chunks = [(0, ch) for ch in HEAD_CHUNKS] + [(1, ch) for ch in HEAD_CHUNKS]
        pci = 0

        def emit_pre(upto):
            nonlocal pci
            while pci < min(upto, len(pre_chunks)):
                hp, (s, c0, c1) = pre_chunks[pci]
                sbuf = mmps.tile([128, 1024], F32, tag="mmps", name=f"pre{pci}")
                exp_chunk(hp, s, c0, c1, sbuf[:].rearrange("p (e w) -> p e w", e=2))
                pci += 1

        for tt in range(NT):
            ps = mmps.tile([128, 1024], F32, tag="mmps", name=f"v{tt}")[:, 0:512]
            for kc in range(NKC):
                nc.tensor.matmul(
                    ps,
                    xT_sb[:, tt // 4, kc, (tt % 4) * 128 : (tt % 4 + 1) * 128],
                    wv_sb[:, kc, :],
                    start=(kc == 0),
                    stop=(kc == NKC - 1),
                )
            nc.vector.tensor_copy(
                out=v_sb[:, tt, :, 0:DV],
                in_=ps[:].rearrange("p (h d) -> p h d", h=HPG),
            )
            emit_pre(3 * (tt + 1))
        emit_pre(len(pre_chunks))

    # ---------------- phase 3: differential attention ----------------
    with (
        tc.tile_pool(name="sps", bufs=1, space="PSUM") as s_pool,
        tc.tile_pool(name="ups", bufs=1, space="PSUM") as u_pool,
        tc.tile_pool(name="comb", bufs=2) as comb,
        tc.tile_pool(name="ohp", bufs=1, space="SBUF") as ohp,
    ):
        oh_tiles = {}
        # per-(h,tj) rsqrt input: ssq/64 + eps*den0^2
        ssq_all = ohp.tile([128, HPG, NT], F32, tag="ssqall", name="ssqall")
        sb_idx = [0]

        def next_sbuf(name):
            t = s_pool.tile(
                [128, 2, 512], F32, tag=f"sh{sb_idx[0] % 2}", name=name
            )
            sb_idx[0] += 1
            return t

        def emit_av(h, s, u_big):
            # U[t-block, dv|den] += expS^T[s-block, t-block].T @ v_aug[s-block]
            es_t = es_store[(h, s)]
            for e in range(2):
                for tj in range(s, NT):
                    nc.tensor.matmul(
                        u_big[:, e, tj, 0 : DV + 1],
                        es_t[:, e, (tj - s) * 128 : (tj - s + 1) * 128],
                        v_sb[:, s, h, 0 : DV + 1],
                        start=(s == 0 and tj % 4 == 0),
                        stop=(s == tj and tj % 4 == 3),
                    )

        # ---- RMS in sub-batches so outcat rows transpose while later heads
        # still compute; apply-muls go on gpsimd to keep DVE free for the
        # next head's epilogue (which gates its AV matmuls via u_big).
        def emit_rms(h_lo, h_hi, dma_eng):
            w = (h_hi - h_lo) * NT
            rstd = comb.tile([128, w], F32, tag=f"rstd{h_lo}", name=f"rstd{h_lo}")
            nc.scalar.activation(
                out=rstd, in_=ssq_all[:, h_lo:h_hi, :], func=Act.Sqrt
            )
            nc.vector.reciprocal(out=rstd, in_=rstd)
            for h in range(h_lo, h_hi):
                c0 = (h - h_lo) * NT
                o0 = (h % 2) * DV
                nc.gpsimd.tensor_mul(
                    outcat_sb[:, h // 2, :, o0 : o0 + DV],
                    oh_tiles[h],
                    rstd[:, c0 : c0 + NT][:, :, None].broadcast_to([128, NT, DV]),
                )
            # one transpose per 128-channel (head-pair) row chunk: the DMA
            # transpose maps in[t, tj, ch] -> out[ch, tj, t]
            for iz in range(h_lo // 2, h_hi // 2):
                dma_eng.dma_start_transpose(
                    out=outcatT_sb[:, iz, :].rearrange("p (tj t) -> p tj t", tj=NT),
                    in_=outcat_sb[:, iz, :, :],
                )

        for h in range(HPG):
            u_big = u_pool.tile([128, 2, NT, 128], F32, tag="ub", name=f"u_{h}")
            # producer for head h+2 rides along in this head's AV slot
            hp = h + 2
            np_ch = NCH if hp < HPG else 0
            ci = 0

            def emit_prod(upto):
                nonlocal ci
                while ci < min(upto, np_ch):
                    s, c0, c1 = HEAD_CHUNKS[ci]
                    exp_chunk(hp, s, c0, c1, next_sbuf(f"s{hp}_{ci}"))
                    ci += 1

            for s in range(NT):
                # front-load 3 chunks: they fill the PE while the previous
                # head's epilogue still holds u_big (WAR on the first AV)
                emit_prod(3 + s * (NCH - 3) // (NT - 1))
                emit_av(h, s, u_big)
            emit_prod(np_ch)

            # ---- batched normalize/combine epilogue --------------------
            # oh = U0/den0 - lam*U1/den1; RMS norm is scale-invariant, so
            # normalize(oh) == normalize(U0 - lam'*U1), lam' = lam*den0/den1,
            # with the eps term exact via bias = eps*den0^2:
            #   oh/rms(oh) = oh' * rsqrt(mean(oh'^2) + eps*den0^2)
            # one copy releases u_big for the next head's AVs (the producer
            # chunks emitted above keep the PE fed while it drains)
            ucp = comb.tile([128, 2, NT, DV + 1], F32, tag="ucp", name=f"ucp{h}")
            nc.vector.tensor_copy(out=ucp, in_=u_big[:, :, :, 0 : DV + 1])
            ub = ucp[:]  # [128, 2, NT, 65] in SBUF
            U0 = ub[:, 0, :, 0:DV]
            U1 = ub[:, 1, :, 0:DV]
            d0 = ub[:, 0, :, DV : DV + 1]
            d1 = ub[:, 1, :, DV : DV + 1]
            rden = comb.tile([128, NT], F32, tag="rd", name=f"rd{h}")
            nc.vector.reciprocal(out=rden, in_=d1)
            d0s = comb.tile([128, NT], F32, tag="d0", name=f"d0{h}")
            nc.vector.tensor_scalar(
                out=d0s, in0=d0, scalar1=math.sqrt(EPS), scalar2=None, op0=AluOp.mult
            )
            lamp = comb.tile([128, NT], F32, tag="lp", name=f"lp{h}")
            nc.vector.scalar_tensor_tensor(
                out=lamp,
                in0=d0s,
                scalar=lam_sb[:],
                in1=rden,
                op0=AluOp.mult,
                op1=AluOp.mult,
            )
            tmp = comb.tile([128, NT, DV], F32, tag="tmp", name=f"tmp{h}")
            nc.vector.tensor_mul(
                tmp, U1, lamp[:, :, None].broadcast_to([128, NT, DV])
            )
            oh = ohp.tile([128, NT, DV], F32, tag=f"oh{h}", name=f"oh{h}")
            nc.vector.tensor_sub(oh, U0, tmp)
            sq = comb.tile([128, NT, DV], F32, tag="sq", name=f"sq{h}")
            nc.vector.tensor_mul(sq, oh, oh)
            nc.vector.tensor_reduce(
                out=ssq_all[:, h, :], in_=sq, axis=mybir.AxisListType.X, op=AluOp.add
            )
            bias = comb.tile([128, NT], F32, tag="bi", name=f"bi{h}")
            nc.vector.tensor_mul(bias, d0s, d0s)  # = eps*den0^2
            nc.vector.scalar_tensor_tensor(
                out=ssq_all[:, h, :],
                in0=ssq_all[:, h, :],
                scalar=1.0 / DV,
                in1=bias,
                op0=AluOp.mult,
                op1=AluOp.add,
            )
            oh_tiles[h] = oh
            if h == 3:
                emit_rms(0, 4, nc.sync)
            elif h == 5:
                emit_rms(4, 6, nc.sync)

        emit_rms(6, HPG, nc.scalar)

    # ---------------- phase 4+5: output projection ----------------
    with (
        tc.tile_pool(name="pps", bufs=4, space="PSUM") as pps,
        tc.tile_pool(name="yout", bufs=2) as yout,
    ):
        for tt in range(NT):
            yt = yout.tile([128, C], F16, tag="yt", name=f"y{tt}")
            for nh in range(2):
                ps = pps.tile([128, 512], F32, tag="pp", name=f"pp{tt}{nh}")
                for rr in range(4):
                    nc.tensor.matmul(
                        ps,
                        outcatT_sb[:, rr, tt * 128 : (tt + 1) * 128],
                        wp_sb[:, rr, nh * 512 : (nh + 1) * 512],
                        start=(rr == 0),
                        stop=(rr == 3),
                    )
                nc.vector.tensor_copy(out=yt[:, nh * 512 : (nh + 1) * 512], in_=ps)
            nc.sync.dma_start(out=y[tt * 128 : (tt + 1) * 128, :], in_=yt)


def build_nc():
    nc = bass.Bass()
    # host pre-transposed, per-partition contiguous layouts (see make_in_maps)
    xT = nc.declare_dram_parameter("xT", [128, 2, NKC, 512], F16, isOutput=False)
    w_qk = nc.declare_dram_parameter("w_qk", [128, 8, NKC, 128], F16, isOutput=False)
    w_v = nc.declare_dram_parameter("w_v", [128, NKC, 512], F16, isOutput=False)
    w_p = nc.declare_dram_parameter("w_p", [128, 4, C], F16, isOutput=False)
    lam = nc.declare_dram_parameter("lam", [128, 1], F32, isOutput=False)
    y = nc.declare_dram_parameter("y", [T, C], F16, isOutput=True)
    with tile.TileContext(nc) as tc:
        with ExitStack() as ctx:
            _emit(ctx, tc, xT, w_qk, w_v, w_p, lam, y)
    return nc


_NC = None


def _get_nc():
    global _NC
    if _NC is None:
        _NC = build_nc()
    return _NC


def make_in_maps(x, w_attn, w_proj, lambda_q1, lambda_q2, lambda_k1, lambda_k2, gamma):
    x = np.asarray(x, np.float32)
    w_attn = np.asarray(w_attn, np.float32)
    w_proj = np.asarray(w_proj, np.float32)
    lam1 = np.exp(np.sum(np.float32(lambda_q1) * np.float32(lambda_k1), dtype=np.float32))
    lam2 = np.exp(np.sum(np.float32(lambda_q2) * np.float32(lambda_k2), dtype=np.float32))
    lam_full = np.float32(lam1 - lam2 + LAMBDA_INIT)
    # kernel multiplies lam by d0s = sqrt(eps)*den0, so pre-divide here
    lam_tile = np.full((128, 1), lam_full / np.float32(math.sqrt(EPS)), np.float32)
    # fold gamma * (1 - lambda_init) into w_proj rows
    scale = np.tile(np.asarray(gamma, np.float32), H_TOT) * np.float32(1.0 - LAMBDA_INIT)
    w_p_full = (w_proj * scale[:, None]).astype(np.float16)

    in_maps = []
    for core in range(N_CORES):
        b, g = core // G, core % G
        # per-partition contiguous DRAM layouts matching the SBUF tiles
        xt = x[b].T.astype(np.float16)  # [C, T]
        xt4 = xt.reshape(NKC, 128, 2, 512).transpose(1, 2, 0, 3)
        wqk = np.concatenate(
            [
                w_attn[:, g * 512 : (g + 1) * 512],
                w_attn[:, C + g * 512 : C + (g + 1) * 512],
            ],
            axis=1,
        ).astype(np.float16)  # [C, COLS]
        wqk4 = wqk.reshape(NKC, 128, 8, 128).transpose(1, 2, 0, 3)
        wv = w_attn[:, 2 * C + g * 512 : 2 * C + (g + 1) * 512].astype(np.float16)
        wv3 = wv.reshape(NKC, 128, 512).transpose(1, 0, 2)
        wp3 = w_p_full[g * 512 : (g + 1) * 512, :].reshape(4, 128, C).transpose(1, 0, 2)
        in_maps.append(
            {
                "xT": np.ascontiguousarray(xt4),
                "w_qk": np.ascontiguousarray(wqk4),
                "w_v": np.ascontiguousarray(wv3),
                "w_p": np.ascontiguousarray(wp3),
                "lam": lam_tile,
            }
        )
    return in_maps


def assemble(results):
    y = np.empty((B, T, C), np.float32)
    for b in range(B):
        y[b] = results[b * G]["y"].astype(np.float32) + results[b * G + 1][
            "y"
        ].astype(np.float32)
    return y


def kernel(**inputs) -> np.ndarray:
    nc = _get_nc()
    in_maps = make_in_maps(**inputs)
    res = run_bass_kernel_spmd(nc, in_maps, list(range(N_CORES)))
    return assemble(res.results)


# revision 22
# speedup vs baseline: 1.4376x; 1.0007x over previous
"""MultiHeadDiffAttn Trainium2 kernel (v4, lookahead producer/consumer).

Sharding: 8 cores = 4-way data parallel over batch x 2-way tensor parallel
over heads (8 v-heads / 16 half-heads per core).  Each core computes its
batch's qkv projection restricted to its head group, differential attention
with per-half-head softmax, head RMS norm, and a partial output projection
(its 512 rows of w_proj).  Host sums the two partial projections per batch.

Structure (what the measured traces drove):
  - ACT (exp) is the attention-phase bottleneck while the qkv phase leaves
    it idle, and a single S buffer serializes S(s+1) behind exp(s) (WAR).
    So S/exp/mask production is decoupled from AV consumption: S goes
    through two half-size PSUM buffers (2x [128,2,512], chunks alternate),
    exp writes ragged es tiles (only cols [t0:T]), and the producer for
    head h+2 is interleaved into head h's AV slot.  Heads 0-1 produce
    entirely inside the qkv phase (interleaved with the v-projection), so
    ACT chews ~20us of exp while the tensor engine is busy with GEMMs.
  - exp and the causal affine_select run e-batched over one [128, 2, w]
    AP covering both half-heads, halving ACT/GpSimd fixed overhead.
  - The per-(head,tj) normalize/combine epilogue is reworked to
    oh = U0 - lam'*U1 with lam' = lam*den0/den1 (RMS norm is
    scale-invariant; eps is kept exact via bias = eps*den0^2) and batched
    over all 8 tj of a head with stride-0 broadcast APs: ~10 DVE ops per
    head instead of ~40.
  - U accumulates in one [128,2,8,128] PSUM tile (4 banks); with the two
    S halves (4 banks) PSUM is exactly full.
  - q chunks scatter straight from PSUM into qTp (the q half of qkT was
    never read); input DMA issue is spread across sync/scalar queues and
    ordered so the first matmul's operands land first; qTp zero-fill is
    split across vector/gpsimd.
  - y is stored f16 (host sums partials in f32) on the scalar queue;
    outcat transposes go per-RMS-batch (heads 0-3 transpose while heads
    4-7 still compute).
"""

import math
from contextlib import ExitStack

import numpy as np

import concourse.bass as bass
import concourse.tile as tile
from concourse import mybir
from concourse.bass_utils import run_bass_kernel_spmd

# The deployed walrus rejects instructions carrying more than one sync wait
# ("Too many sync wait commands" in setupSyncWait).  Legalize at the BIR-JSON
# level: for every instruction with >1 wait, hoist the extra waits onto NoOp
# instructions inserted just before it on the same engine (engine streams are
# in-order, so semantics are identical).
_MAX_WAITS = 1


def _legalize_sync_waits(d):
    for f in d.get("functions", []):
        for bb in f.get("blocks", []):
            out = []
            for inst in bb["instructions"]:
                si = inst.get("sync_info")
                waits = (si or {}).get("on_wait") or []
                if len(waits) > _MAX_WAITS:
                    extra = waits[: len(waits) - _MAX_WAITS]
                    keep = waits[len(waits) - _MAX_WAITS :]
                    for j in range(0, len(extra), _MAX_WAITS):
                        nop = {
                            "engine": inst["engine"],
                            "ins": [],
                            "outs": [],
                            "name": f"{inst['name']}-lw{j}",
                            "opcode": "NoOp",
                            "sync_info": {
                                "on_wait": extra[j : j + _MAX_WAITS],
                                "on_update": [],
                            },
                        }
                        if "debug" in inst:
                            nop["debug"] = inst["debug"]
                        out.append(nop)
                    si["on_wait"] = keep
                out.append(inst)
            bb["instructions"] = out
    return d


_orig_to_json_bytes = bass.Bass.to_json_bytes


def _patched_to_json_bytes(self, *a, **kw):
    import json as _json

    raw = _orig_to_json_bytes(self, *a, **kw)
    return _json.dumps(_legalize_sync_waits(_json.loads(raw))).encode()


bass.Bass.to_json_bytes = _patched_to_json_bytes

F32 = mybir.dt.float32
F16 = mybir.dt.float16

B, T, C = 4, 1024, 1024
H_TOT = 16  # total v-heads
HD = 32  # half-head dim
DV = 64  # v-head dim
G = 2  # head groups (tensor parallel)
HPG = H_TOT // G  # 8 v-heads per core
COLS = 1024  # q cols + k cols per group
LAMBDA_INIT = 0.8 - 0.6 * math.exp(-0.3 * (1 - 1))  # 0.2
EPS = 1e-5
N_CORES = 8

NT = T // 128  # 8 t-tiles
NKC = C // 128  # 8 contraction chunks

# S/exp chunks per head: column ranges sized to fit a [128,2,512] PSUM
# half-buffer (per-e region stays inside one bank).
HEAD_CHUNKS = []
for _s in range(NT):
    _t0 = 128 * _s
    if _s < 4:
        HEAD_CHUNKS.append((_s, _t0, 512))
        HEAD_CHUNKS.append((_s, 512, 1024))
    else:
        HEAD_CHUNKS.append((_s, _t0, 1024))
NCH = len(HEAD_CHUNKS)  # 12


def _emit(ctx: ExitStack, tc: tile.TileContext, xT, w_qk, w_v, w_p, lam, y):
    nc = tc.nc
    AluOp = mybir.AluOpType
    Act = mybir.ActivationFunctionType

    const = ctx.enter_context(tc.tile_pool(name="const", bufs=1))
    lam_sb = const.tile([128, 1], F32)
    nc.sync.dma_start(out=lam_sb, in_=lam[:])

    big = ctx.enter_context(tc.tile_pool(name="big", bufs=1))
    kT_sb = big.tile([128, 4, T], F16)  # row-chunks of k^T [512, T]
    v_sb = big.tile([128, NT, HPG, 128], F16)  # [s-chunk][head][dv | ones]
    # [head-pair chunk][t-chunk][128 ch]: per-chunk contiguous so one DMA
    # transpose handles all 8 t-chunks of a head pair
    outcat_sb = big.tile([128, 4, NT, 128], F16)
    outcatT_sb = big.tile([128, 4, T], F16)  # row-chunks of [512, T]
    wp_sb = big.tile([128, 4, C], F16)
    # per-half-head q, zero-padded to K=128: data lives at the same 32-row
    # strip as that half-head's k rows inside its kT chunk, so the S^T
    # matmul can contract over the full 128 partitions at full stream rate
    # (the other half-heads' k rows meet zero q rows).
    qTp_sb = big.tile([128, 2 * HPG, T], F16)

    nc.vector.memset(qTp_sb[:, 0:8, :], 0.0)
    nc.gpsimd.memset(qTp_sb[:, 8:16, :], 0.0)
    nc.gpsimd.memset(v_sb[:, :, :, DV : DV + 1], 1.0)  # softmax-denominator col
    # ragged es tiles: tag per s-block, two generations deep
    es_pool = ctx.enter_context(tc.tile_pool(name="es", bufs=2))
    es_store = {}  # (h, s) -> tile [128, 2, T - 128*s]

    def exp_chunk(h, s, c0, c1, sbuf):
        """S matmuls for both e of chunk [c0:c1) into sbuf, fused exp into
        the ragged es tile, causal mask after the diagonal chunk."""
        t0 = 128 * s
        if c0 == t0:
            es_t = es_pool.tile([128, 2, T - t0], F16, tag=f"es{s}", name=f"es{h}_{s}")
            es_store[(h, s)] = es_t
        else:
            es_t = es_store[(h, s)]
        base = 0 if c0 < 512 else 512
        kc_ = h // 2
        for e in range(2):
            nc.tensor.matmul(
                sbuf[:, e, c0 - base : c1 - base],
                kT_sb[:, kc_, t0 : t0 + 128],
                qTp_sb[:, 2 * h + e, c0:c1],
                start=True,
                stop=True,
            )
        nc.scalar.activation(
            out=es_t[:, :, c0 - t0 : c1 - t0],
            in_=sbuf[:, :, c0 - base : c1 - base],
            func=Act.Exp,
            scale=1.0 / 32.0,
        )
        if c0 == t0:  # diagonal block: keep t >= s for both half-heads
            nc.gpsimd.affine_select(
                out=es_t[:, :, 0:128],
                in_=es_t[:, :, 0:128],
                pattern=[[0, 2], [1, 128]],
                compare_op=AluOp.is_ge,
                fill=0.0,
                base=0,
                channel_multiplier=-1,
            )

    # ---------------- phase 1+2: qkv projections ----------------
    with (
        tc.tile_pool(name="xw", bufs=1) as xw,
        tc.tile_pool(name="mmps", bufs=4, space="PSUM") as mmps,
    ):
        # host pre-transposes everything into SBUF-ready, per-partition
        # contiguous layouts so each DMA is a handful of big descriptors
        # instead of ~1k strided 256B packets.
        xT_sb = xw.tile([128, 2, NKC, 512], F16)  # [nh][kc][cols]
        wqk_sb = xw.tile([128, 8, NKC, 128], F16)  # [cc][kc][cols]
        wv_sb = xw.tile([128, NKC, 512], F16)

        def load_wqk(cc):
            nc.sync.dma_start(out=wqk_sb[:, cc, :, :], in_=w_qk[:, cc, :, :])

        # order feeds the first matmuls first; xT/wv/wp issue on the scalar
        # queue so they don't serialize behind the wqk issues on sync.
        load_wqk(0)
        nc.scalar.dma_start(out=xT_sb[:, 0, :, :], in_=xT[:, 0, :, :])
        load_wqk(1)
        load_wqk(2)
        nc.scalar.dma_start(out=xT_sb[:, 1, :, :], in_=xT[:, 1, :, :])
        for cc in range(3, 8):
            load_wqk(cc)
        nc.scalar.dma_start(out=wv_sb, in_=w_v[:])
        nc.scalar.dma_start(out=wp_sb, in_=w_p[:])

        # chunk cc of [q^T; k^T] = w_qk[:, cc-block].T @ x^T
        for cc in range(8):
            for nh in range(2):
                ps = mmps.tile([128, 1024], F32, tag="mmps", name=f"qk{cc}{nh}")[:, 0:512]
                for kc in range(NKC):
                    nc.tensor.matmul(
                        ps,
                        wqk_sb[:, cc, kc, :],
                        xT_sb[:, nh, kc, :],
                        start=(kc == 0),
                        stop=(kc == NKC - 1),
                    )
                if cc < 4:  # q chunk: scatter the 4 half-heads into qTp
                    # split between DVE and the (idle-until-prebake) ACT
                    for j in range(4):
                        hh = cc * 4 + j
                        dst = qTp_sb[
                            j * 32 : (j + 1) * 32, hh, nh * 512 : (nh + 1) * 512
                        ]
                        src = ps[j * 32 : (j + 1) * 32, :]
                        if j < 2:
                            nc.vector.tensor_copy(out=dst, in_=src)
                        else:
                            nc.scalar.copy(out=dst, in_=src)
                else:  # k chunk
                    nc.vector.tensor_copy(
                        out=kT_sb[:, cc - 4, nh * 512 : (nh + 1) * 512], in_=ps
                    )

        # v projection interleaved with the full S/exp production for heads
        # 0-1 (24 chunks): ACT chews exp while the PE does the v GEMMs.
        pre_chunks = [(0, ch) for ch in HEAD_CHUNKS] + [(1, ch) for ch in HEAD_CHUNKS]
        pci = 0

        def emit_pre(upto):
            nonlocal pci
            while pci < min(upto, len(pre_chunks)):
                hp, (s, c0, c1) = pre_chunks[pci]
                sbuf = mmps.tile([128, 1024], F32, tag="mmps", name=f"pre{pci}")
                exp_chunk(hp, s, c0, c1, sbuf[:].rearrange("p (e w) -> p e w", e=2))
                pci += 1

        for tt in range(NT):
            ps = mmps.tile([128, 1024], F32, tag="mmps", name=f"v{tt}")[:, 0:512]
            for kc in range(NKC):
                nc.tensor.matmul(
                    ps,
                    xT_sb[:, tt // 4, kc, (tt % 4) * 128 : (tt % 4 + 1) * 128],
                    wv_sb[:, kc, :],
                    start=(kc == 0),
                    stop=(kc == NKC - 1),
                )
            nc.vector.tensor_copy(
                out=v_sb[:, tt, :, 0:DV],
                in_=ps[:].rearrange("p (h d) -> p h d", h=HPG),
            )
            emit_pre(3 * (tt + 1))
        emit_pre(len(pre_chunks))

    # ---------------- phase 3: differential attention ----------------
    with (
        tc.tile_pool(name="sps", bufs=1, space="PSUM") as s_pool,
        tc.tile_pool(name="ups", bufs=1, space="PSUM") as u_pool,
        tc.tile_pool(name="comb", bufs=2) as comb,
        tc.tile_pool(name="ohp", bufs=1, space="SBUF") as ohp,
    ):
        oh_tiles = {}
        # per-(h,tj) rsqrt input: ssq/64 + eps*den0^2
        ssq_all = ohp.tile([128, HPG, NT], F32, tag="ssqall", name="ssqall")
        sb_idx = [0]

        def next_sbuf(name):
            t = s_pool.tile(
                [128, 2, 512], F32, tag=f"sh{sb_idx[0] % 2}", name=name
            )
            sb_idx[0] += 1
            return t

        def emit_av(h, s, u_big):
            # U[t-block, dv|den] += expS^T[s-block, t-block].T @ v_aug[s-block]
            es_t = es_store[(h, s)]
            for e in range(2):
                for tj in range(s, NT):
                    nc.tensor.matmul(
                        u_big[:, e, tj, 0 : DV + 1],
                        es_t[:, e, (tj - s) * 128 : (tj - s + 1) * 128],
                        v_sb[:, s, h, 0 : DV + 1],
                        start=(s == 0 and tj % 4 == 0),
                        stop=(s == tj and tj % 4 == 3),
                    )

        # ---- RMS in sub-batches so outcat rows transpose while later heads
        # still compute; apply-muls go on gpsimd to keep DVE free for the
        # next head's epilogue (which gates its AV matmuls via u_big).
        def emit_rms(h_lo, h_hi, dma_eng):
            # rsqrt on DVE (bit-trick seed + 2 Newton steps): a Sqrt on the
            # scalar engine would force two 1.3us ACT_TABLE_LOADs (Exp<->Sqrt)
            # and stall the exp stream it is pipelined with.
            w = (h_hi - h_lo) * NT
            v = ssq_all[:, h_lo:h_hi, :]
            I32 = mybir.dt.int32
            iu = comb.tile([128, w], I32, tag="rsqi", name=f"rsqi{h_lo}")
            nc.vector.tensor_scalar(
                out=iu,
                in0=v.bitcast(I32),
                scalar1=1,
                scalar2=None,
                op0=AluOp.logical_shift_right,
            )
            nc.vector.tensor_scalar(
                out=iu,
                in0=iu,
                scalar1=-1,
                scalar2=0x5F3759DF,
                op0=AluOp.mult,
                op1=AluOp.add,
            )
            y0 = iu[:].bitcast(F32)
            rstd = comb.tile([128, w], F32, tag=f"rstd{h_lo}", name=f"rstd{h_lo}")
            t_ = comb.tile([128, w], F32, tag="rsqt", name=f"rsqt{h_lo}")
            ycur = y0
            for it in range(2):
                nc.vector.tensor_mul(t_, ycur, ycur)
                nc.vector.scalar_tensor_tensor(
                    out=t_, in0=t_, scalar=-0.5, in1=v, op0=AluOp.mult, op1=AluOp.mult
                )
                nc.vector.tensor_scalar(
                    out=t_, in0=t_, scalar1=1.5, scalar2=None, op0=AluOp.add
                )
                nc.vector.tensor_mul(rstd, ycur, t_)
                ycur = rstd[:]
            for h in range(h_lo, h_hi):
                c0 = (h - h_lo) * NT
                o0 = (h % 2) * DV
                nc.gpsimd.tensor_mul(
                    outcat_sb[:, h // 2, :, o0 : o0 + DV],
                    oh_tiles[h],
                    rstd[:, c0 : c0 + NT][:, :, None].broadcast_to([128, NT, DV]),
                )
            # one transpose per 128-channel (head-pair) row chunk: the DMA
            # transpose maps in[t, tj, ch] -> out[ch, tj, t]
            for iz in range(h_lo // 2, h_hi // 2):
                dma_eng.dma_start_transpose(
                    out=outcatT_sb[:, iz, :].rearrange("p (tj t) -> p tj t", tj=NT),
                    in_=outcat_sb[:, iz, :, :],
                )

        for h in range(HPG):
            u_big = u_pool.tile([128, 2, NT, 128], F32, tag="ub", name=f"u_{h}")
            # producer for head h+2 rides along in this head's AV slot
            hp = h + 2
            np_ch = NCH if hp < HPG else 0
            ci = 0

            def emit_prod(upto):
                nonlocal ci
                while ci < min(upto, np_ch):
                    s, c0, c1 = HEAD_CHUNKS[ci]
                    exp_chunk(hp, s, c0, c1, next_sbuf(f"s{hp}_{ci}"))
                    ci += 1

            for s in range(NT):
                # front-load 3 chunks: they fill the PE while the previous
                # head's epilogue still holds u_big (WAR on the first AV)
                emit_prod(3 + s * (NCH - 3) // (NT - 1))
                emit_av(h, s, u_big)
            emit_prod(np_ch)

            # ---- batched normalize/combine epilogue --------------------
            # oh = U0/den0 - lam*U1/den1; RMS norm is scale-invariant, so
            # normalize(oh) == normalize(U0 - lam'*U1), lam' = lam*den0/den1,
            # with the eps term exact via bias = eps*den0^2:
            #   oh/rms(oh) = oh' * rsqrt(mean(oh'^2) + eps*den0^2)
            # one copy releases u_big for the next head's AVs (the producer
            # chunks emitted above keep the PE fed while it drains)
            ucp = comb.tile([128, 2, NT, DV + 1], F32, tag="ucp", name=f"ucp{h}")
            nc.vector.tensor_copy(out=ucp, in_=u_big[:, :, :, 0 : DV + 1])
            ub = ucp[:]  # [128, 2, NT, 65] in SBUF
            U0 = ub[:, 0, :, 0:DV]
            U1 = ub[:, 1, :, 0:DV]
            d0 = ub[:, 0, :, DV : DV + 1]
            d1 = ub[:, 1, :, DV : DV + 1]
            rden = comb.tile([128, NT], F32, tag="rd", name=f"rd{h}")
            nc.vector.reciprocal(out=rden, in_=d1)
            d0s = comb.tile([128, NT], F32, tag="d0", name=f"d0{h}")
            nc.vector.tensor_scalar(
                out=d0s, in0=d0, scalar1=math.sqrt(EPS), scalar2=None, op0=AluOp.mult
            )
            lamp = comb.tile([128, NT], F32, tag="lp", name=f"lp{h}")
            nc.vector.scalar_tensor_tensor(
                out=lamp,
                in0=d0s,
                scalar=lam_sb[:],
                in1=rden,
                op0=AluOp.mult,
                op1=AluOp.mult,
            )
            tmp = comb.tile([128, NT, DV], F32, tag="tmp", name=f"tmp{h}")
            nc.vector.tensor_mul(
                tmp, U1, lamp[:, :, None].broadcast_to([128, NT, DV])
            )
            oh = ohp.tile([128, NT, DV], F32, tag=f"oh{h}", name=f"oh{h}")
            nc.vector.tensor_sub(oh, U0, tmp)
            sq = comb.tile([128, NT, DV], F32, tag="sq", name=f"sq{h}")
            nc.vector.tensor_mul(sq, oh, oh)
            nc.vector.tensor_reduce(
                out=ssq_all[:, h, :], in_=sq, axis=mybir.AxisListType.X, op=AluOp.add
            )
            bias = comb.tile([128, NT], F32, tag="bi", name=f"bi{h}")
            nc.vector.tensor_mul(bias, d0s, d0s)  # = eps*den0^2
            nc.vector.scalar_tensor_tensor(
                out=ssq_all[:, h, :],
                in0=ssq_all[:, h, :],
                scalar=1.0 / DV,
                in1=bias,
                op0=AluOp.mult,
                op1=AluOp.add,
            )
            oh_tiles[h] = oh
            if h == 3:
                emit_rms(0, 4, nc.sync)
            elif h == 5:
                emit_rms(4, 6, nc.sync)

        emit_rms(6, HPG, nc.scalar)

    # ---------------- phase 4+5: output projection ----------------
    with (
        tc.tile_pool(name="pps", bufs=4, space="PSUM") as pps,
        tc.tile_pool(name="yout", bufs=2) as yout,
    ):
        for tt in range(NT):
            yt = yout.tile([128, C], F16, tag="yt", name=f"y{tt}")
            for nh in range(2):
                ps = pps.tile([128, 512], F32, tag="pp", name=f"pp{tt}{nh}")
                for rr in range(4):
                    nc.tensor.matmul(
                        ps,
                        outcatT_sb[:, rr, tt * 128 : (tt + 1) * 128],
                        wp_sb[:, rr, nh * 512 : (nh + 1) * 512],
                        start=(rr == 0),
                        stop=(rr == 3),
                    )
                nc.vector.tensor_copy(out=yt[:, nh * 512 : (nh + 1) * 512], in_=ps)
            nc.sync.dma_start(out=y[tt * 128 : (tt + 1) * 128, :], in_=yt)


def build_nc():
    nc = bass.Bass()
    # host pre-transposed, per-partition contiguous layouts (see make_in_maps)
    xT = nc.declare_dram_parameter("xT", [128, 2, NKC, 512], F16, isOutput=False)
    w_qk = nc.declare_dram_parameter("w_qk", [128, 8, NKC, 128], F16, isOutput=False)
    w_v = nc.declare_dram_parameter("w_v", [128, NKC, 512], F16, isOutput=False)
    w_p = nc.declare_dram_parameter("w_p", [128, 4, C], F16, isOutput=False)
    lam = nc.declare_dram_parameter("lam", [128, 1], F32, isOutput=False)
    y = nc.declare_dram_parameter("y", [T, C], F16, isOutput=True)
    with tile.TileContext(nc) as tc:
        with ExitStack() as ctx:
            _emit(ctx, tc, xT, w_qk, w_v, w_p, lam, y)
    return nc


_NC = None


def _get_nc():
    global _NC
    if _NC is None:
        _NC = build_nc()
    return _NC


def make_in_maps(x, w_attn, w_proj, lambda_q1, lambda_q2, lambda_k1, lambda_k2, gamma):
    x = np.asarray(x, np.float32)
    w_attn = np.asarray(w_attn, np.float32)
    w_proj = np.asarray(w_proj, np.float32)
    lam1 = np.exp(np.sum(np.float32(lambda_q1) * np.float32(lambda_k1), dtype=np.float32))
    lam2 = np.exp(np.sum(np.float32(lambda_q2) * np.float32(lambda_k2), dtype=np.float32))
    lam_full = np.float32(lam1 - lam2 + LAMBDA_INIT)
    # kernel multiplies lam by d0s = sqrt(eps)*den0, so pre-divide here
    lam_tile = np.full((128, 1), lam_full / np.float32(math.sqrt(EPS)), np.float32)
    # fold gamma * (1 - lambda_init) into w_proj rows
    scale = np.tile(np.asarray(gamma, np.float32), H_TOT) * np.float32(1.0 - LAMBDA_INIT)
    w_p_full = (w_proj * scale[:, None]).astype(np.float16)

    in_maps = []
    for core in range(N_CORES):
        b, g = core // G, core % G
        # per-partition contiguous DRAM layouts matching the SBUF tiles
        xt = x[b].T.astype(np.float16)  # [C, T]
        xt4 = xt.reshape(NKC, 128, 2, 512).transpose(1, 2, 0, 3)
        wqk = np.concatenate(
            [
                w_attn[:, g * 512 : (g + 1) * 512],
                w_attn[:, C + g * 512 : C + (g + 1) * 512],
            ],
            axis=1,
        ).astype(np.float16)  # [C, COLS]
        wqk4 = wqk.reshape(NKC, 128, 8, 128).transpose(1, 2, 0, 3)
        wv = w_attn[:, 2 * C + g * 512 : 2 * C + (g + 1) * 512].astype(np.float16)
        wv3 = wv.reshape(NKC, 128, 512).transpose(1, 0, 2)
        wp3 = w_p_full[g * 512 : (g + 1) * 512, :].reshape(4, 128, C).transpose(1, 0, 2)
        in_maps.append(
            {
                "xT": np.ascontiguousarray(xt4),
                "w_qk": np.ascontiguousarray(wqk4),
                "w_v": np.ascontiguousarray(wv3),
                "w_p": np.ascontiguousarray(wp3),
                "lam": lam_tile,
            }
        )
    return in_maps


def assemble(results):
    y = np.empty((B, T, C), np.float32)
    for b in range(B):
        y[b] = results[b * G]["y"].astype(np.float32) + results[b * G + 1][
            "y"
        ].astype(np.float32)
    return y


def kernel(**inputs) -> np.ndarray:
    nc = _get_nc()
    in_maps = make_in_maps(**inputs)
    res = run_bass_kernel_spmd(nc, in_maps, list(range(N_CORES)))
    return assemble(res.results)
